# revision 1
# baseline (speedup 1.0000x reference)
"""Transformer decoder layer (masked self-attn + cross-attn + FFN, 3x LayerNorm)
for Trainium2, data-parallel over batch across 8 NeuronCores.

Per-core problem: L=1024 tokens, D=1024 model dim, H=16 heads x 64, DFF=4096.
Strategy: bf16 matmuls (fp32 PSUM accumulation), fp32 softmax/LN math.
Activations kept in [L, D] layout for LN/softmax; bf16 [D, L] transposed copies
(DMA xbar transpose) feed the TensorEngine. Attention computes S^T = K Q^T per
head ([Lk, Lq] layout), exp on ScalarE with the 1/sqrt(dk) scale folded in,
then O^T = [V | 1]^T expS^T which yields the softmax denominator as row 64 of
the PSUM output; O^T is DMA-transposed back and normalized with a native
per-partition scalar multiply. Causal masking skips fully-masked 128x512 score
blocks entirely and applies a precomputed 0/1 mask to diagonal blocks.

SBUF uses phase-scoped tile pools on the two LIFO stack sides:
left = residual carriers (x1_bf > x_bf > sa; ca; y), right = matmul operands
and weights (encT > qkv > per-phase pools). Peak ~200 KB/partition.
"""

import sys

sys.path.insert(0, "/opt/trn_rl_repo")

import numpy as np

import concourse.bass as bass
import concourse.mybir as mybir
import concourse.tile as tile
from concourse import bacc
from concourse.bass_utils import run_bass_kernel_spmd

FP32 = mybir.dt.float32
BF16 = mybir.dt.bfloat16
AF = mybir.ActivationFunctionType
ALU = mybir.AluOpType

B = 8
L = 1024
D = 1024
H = 16
DK = 64
DFF = 4096
P = 128
NT = L // P  # 8 l-tiles
DT = D // P  # 8 d-tiles
NP = H // 2  # 8 head pairs
LC = 512  # lq chunk
NLC = L // LC  # 2
FH = 2  # ffn dff halves
FT = DFF // FH // P  # 16 f-tiles per half
EPS = 1e-5

WEIGHT_NAMES = [
    "m_wq", "m_bq", "m_wk", "m_bk", "m_wv", "m_bv",
    "c_wq", "c_bq", "c_wk", "c_bk", "c_wv", "c_bv",
    "ff_w1", "ff_b1", "ff_w2", "ff_b2",
    "ln1_g", "ln1_b", "ln2_g", "ln2_b",
]

INPUT_SPECS = {
    "decoder_embedding": [L, D],
    "encoder_output": [L, D],
    "m_wq": [H, D, DK], "m_bq": [H, DK],
    "m_wk": [H, D, DK], "m_bk": [H, DK],
    "m_wv": [H, D, DK], "m_bv": [H, DK],
    "c_wq": [H, D, DK], "c_bq": [H, DK],
    "c_wk": [H, D, DK], "c_bk": [H, DK],
    "c_wv": [H, D, DK], "c_bv": [H, DK],
    "ff_w1": [D, DFF], "ff_b1": [DFF],
    "ff_w2": [DFF, D], "ff_b2": [D],
    "ln1_g": [D], "ln1_b": [D],
    "ln2_g": [D], "ln2_b": [D],
}


def _bcast_ap(ap, parts=P):
    """Broadcast a 1-D DRAM AP across `parts` partitions (step-0 partition dim)."""
    return bass.AP(tensor=ap.tensor, offset=ap.offset, ap=[[0, parts]] + list(ap.ap))


class Pools:
    """Manual pool open/close; per-side LIFO order is asserted at build time."""

    def __init__(self, tc):
        self.tc = tc
        self.stacks = {"left": [], "right": []}

    def open(self, name, bufs, side="right", space=bass.MemorySpace.SBUF):
        cm = self.tc.tile_pool(name=name, bufs=bufs, side=side, space=space)
        pool = cm.__enter__()
        self.stacks[side].append((name, cm))
        return pool

    def close(self, name):
        for side, stack in self.stacks.items():
            for i, (n, cm) in enumerate(stack):
                if n == name:
                    assert i == len(stack) - 1, (
                        f"pool {name} is not on top of {side} stack: "
                        f"{[x[0] for x in stack]}"
                    )
                    stack.pop()
                    cm.__exit__(None, None, None)
                    return
        raise KeyError(name)

    def close_all(self):
        for side in ("left", "right"):
            while self.stacks[side]:
                _, cm = self.stacks[side].pop()
                cm.__exit__(None, None, None)


def build(stop_after=None, n_bodies=1):
    nc = bacc.Bacc("TRN2", target_bir_lowering=False, debug=False, num_devices=B)

    dram = {}
    for name, shape in INPUT_SPECS.items():
        dram[name] = nc.dram_tensor(name, shape, FP32, kind="ExternalInput")
    out_d = nc.dram_tensor("out", [L, D], FP32, kind="ExternalOutput")

    with tile.TileContext(nc) as tc:
        for _ in range(n_bodies):
            _emit(nc, tc, dram, out_d, stop_after)
    nc.compile()
    return nc


def _emit(nc, tc, dram, out_d, stop_after):
    pl = Pools(tc)
    const = pl.open("const", 1)
    stage = pl.open("stage", 3)
    psum = pl.open("psum", 8, space=bass.MemorySpace.PSUM)

    def ap(name):
        return dram[name].ap()

    # ---- constants ----
    eps_t = const.tile([P, 1], FP32)
    nc.vector.memset(eps_t, EPS)

    # causal 0/1 masks for diagonal blocks: mask[i][kk, qq] = 1 if qq >= kk + i*128
    mask_bf = const.tile([P, 4, LC], BF16)
    for i in range(4):
        m32 = stage.tile([P, LC], FP32, tag="st32")
        nc.vector.memset(m32, 1.0)
        nc.gpsimd.affine_select(
            out=m32,
            in_=m32,
            compare_op=ALU.is_ge,
            fill=0.0,
            base=-(i * P),
            pattern=[[1, LC]],
            channel_multiplier=-1,
        )
        nc.vector.tensor_copy(mask_bf[:, i, :], m32)

    def load_bias_pair_col(name):
        # [H, DK] -> [128, NP]: partition = (h%2)*64 + k, col = h//2
        t = const.tile([P, NP], FP32, tag=f"bc_{name}")
        nc.sync.dma_start(t, ap(name).rearrange("(pr i) k -> (i k) pr", i=2))
        return t

    b1_col = const.tile([P, DFF // P], FP32)
    nc.sync.dma_start(b1_col, ap("ff_b1").rearrange("(ft p) -> p ft", p=P))
    b2_col = const.tile([P, DT], FP32)
    nc.sync.dma_start(b2_col, ap("ff_b2").rearrange("(dt p) -> p dt", p=P))

    # ---- long-lived tiles ----
    # LEFT: residual carriers; RIGHT: matmul operands
    x1res_pool = pl.open("x1res", 1, side="left")
    x1_bf = x1res_pool.tile([P, NT, D], BF16, tag="x1_bf")
    xres_pool = pl.open("xres", 1, side="left")
    x_bf = xres_pool.tile([P, NT, D], BF16, tag="x_bf")

    encT_pool = pl.open("encT", 1)
    encT = encT_pool.tile([P, DT, L], BF16, tag="encT")
    qkv = pl.open("qkv", 1)
    qt_t = qkv.tile([P, NP, L], BF16, tag="qt")
    kt_t = qkv.tile([P, NP, L], BF16, tag="kt")
    vn_t = qkv.tile([P, H, NT, 80], BF16, tag="vn")  # [.., 0:64]=V, 64=ones col
    xT_pool = pl.open("xT", 1)
    xT = xT_pool.tile([P, DT, L], BF16, tag="xT")

    def transpose_into(dstT, src_bf, lt):
        # src_bf [P, D] bf16 (l-tile lt) -> dstT[:, dt, lt*P:(lt+1)*P] per d-tile
        for dt_ in range(DT):
            nc.sync.dma_start(
                dstT[:, dt_, lt * P:(lt + 1) * P],
                src_bf[:, dt_ * P:(dt_ + 1) * P],
                transpose=True,
            )

    for lt in range(NT):
        e32 = stage.tile([P, D], FP32, tag="st32")
        nc.sync.dma_start(e32, ap("encoder_output")[lt * P:(lt + 1) * P, :])
        ebf = stage.tile([P, D], BF16, tag="stbf")
        nc.vector.tensor_copy(ebf, e32)
        transpose_into(encT, ebf, lt)

    for lt in range(NT):
        x32 = stage.tile([P, D], FP32, tag="st32")
        nc.sync.dma_start(x32, ap("decoder_embedding")[lt * P:(lt + 1) * P, :])
        nc.vector.tensor_copy(x_bf[:, lt, :], x32)
        transpose_into(xT, x_bf[:, lt, :], lt)

    # ---- helpers ----
    def load_attn_w(wpool, name):
        # [H, D, DK] -> [128(d), DT, H*DK] bf16; col = h*64+k = pair*128 + (h%2)*64 + k
        w = wpool.tile([P, DT, H * DK], BF16, tag="w_attn")
        for dt_ in range(DT):
            st = stage.tile([P, H, DK], FP32, tag="st32")
            nc.sync.dma_start(
                st, ap(name)[:, dt_ * P:(dt_ + 1) * P, :].rearrange("h d k -> d h k")
            )
            nc.vector.tensor_copy(
                w[:, dt_, :].rearrange("p (h k) -> p h k", h=H), st
            )
        return w

    def project(dst, w, b_col, srcT):
        # dst [P, NP, L] bf16: dst[i*64+k, pr, l] = sum_d srcT[d, l] w[d, pr*128+i*64+k] + b
        for pr in range(NP):
            for lc in range(NLC):
                ps = psum.tile([P, LC], FP32, tag="ps")
                for dt_ in range(DT):
                    nc.tensor.matmul(
                        ps,
                        w[:, dt_, pr * P:(pr + 1) * P],
                        srcT[:, dt_, lc * LC:(lc + 1) * LC],
                        start=(dt_ == 0),
                        stop=(dt_ == DT - 1),
                    )
                nc.vector.tensor_scalar_add(
                    dst[:, pr, lc * LC:(lc + 1) * LC], ps, b_col[:, pr:pr + 1]
                )

    def project_qkv(srcT_q, srcT_kv, wq_n, wk_n, wv_n, bq_n, bk_n, bv_n):
        wpool = pl.open("w_attn", 2)
        vt_pool = pl.open("vt", 1)
        vt = vt_pool.tile([P, NP, L], BF16, tag="vt")
        bq = load_bias_pair_col(bq_n)
        bk = load_bias_pair_col(bk_n)
        bv = load_bias_pair_col(bv_n)
        wq = load_attn_w(wpool, wq_n)
        project(qt_t, wq, bq, srcT_q)
        wk = load_attn_w(wpool, wk_n)
        project(kt_t, wk, bk, srcT_kv)
        wv = load_attn_w(wpool, wv_n)
        project(vt, wv, bv, srcT_kv)
        # V^T packed pairs -> vn [128(lk), H, NT, 80] with ones col 64
        nc.vector.memset(vn_t[:, :, :, 64:65], 1.0)
        for h in range(H):
            pr, i = divmod(h, 2)
            for kt_ in range(NT):
                nc.sync.dma_start(
                    vn_t[:, h, kt_, 0:64],
                    vt[i * 64:(i + 1) * 64, pr, kt_ * P:(kt_ + 1) * P],
                    transpose=True,
                )
        pl.close("vt")
        pl.close("w_attn")

    def attention(out_sa, causal):
        heads = pl.open("heads", 2)
        otrp = pl.open("otr", 3)
        for h in range(H):
            pr, i = divmod(h, 2)
            r0 = i * 64
            expS = heads.tile([P, NT, L], BF16, tag="expS")
            for lc in range(NLC):
                kts = range(4) if (causal and lc == 0) else range(NT)
                for kt_ in kts:
                    sps = psum.tile([P, LC], FP32, tag="ps")
                    nc.tensor.matmul(
                        sps,
                        kt_t[r0:r0 + 64, pr, kt_ * P:(kt_ + 1) * P],
                        qt_t[r0:r0 + 64, pr, lc * LC:(lc + 1) * LC],
                        start=True,
                        stop=True,
                        tile_position=(r0, 0),
                    )
                    dst = expS[:, kt_, lc * LC:(lc + 1) * LC]
                    nc.scalar.activation(dst, sps, AF.Exp, scale=0.125)
                    if causal and kt_ >= 4 * lc:
                        mi = kt_ - 4 * lc
                        nc.vector.tensor_mul(dst, dst, mask_bf[:, mi, :])
            # O^T (rows 0:64) + softmax denominator (row 64)
            ot = heads.tile([96, L], BF16, tag="ot")
            nc.vector.memset(ot[64:96, :], 0.0)
            for lc in range(NLC):
                kts = list(range(4)) if (causal and lc == 0) else list(range(NT))
                ops = psum.tile([P, LC], FP32, tag="ps")
                for j, kt_ in enumerate(kts):
                    nc.tensor.matmul(
                        ops[0:65, :],
                        vn_t[:, h, kt_, 0:65],
                        expS[:, kt_, lc * LC:(lc + 1) * LC],
                        start=(j == 0),
                        stop=(j == len(kts) - 1),
                    )
                nc.vector.tensor_copy(ot[0:65, lc * LC:(lc + 1) * LC], ops[0:65, :])
            otr = otrp.tile([P, NT, 96], BF16, tag="otr")
            for lt in range(NT):
                nc.sync.dma_start(
                    otr[:, lt, :], ot[:, lt * P:(lt + 1) * P], transpose=True
                )
            rcp = heads.tile([P, NT], FP32, tag="rcp")
            for lt in range(NT):
                nc.vector.reciprocal(rcp[:, lt:lt + 1], otr[:, lt, 64:65])
            for lt in range(NT):
                nc.vector.tensor_scalar_mul(
                    out_sa[:, lt, h * 64:(h + 1) * 64],
                    otr[:, lt, 0:64],
                    rcp[:, lt:lt + 1],
                )
        pl.close("otr")
        pl.close("heads")

    def layer_norm(res, dst, g_t, b_t):
        # res [P, D] f32 -> dst [P, D]
        st = stage.tile([P, 2, 6], FP32, tag="bnst")
        for c in range(2):
            nc.vector.bn_stats(st[:, c, :], res[:, c * 512:(c + 1) * 512])
        mv = stage.tile([P, 2], FP32, tag="bnmv")
        nc.vector.bn_aggr(mv, st)
        rs = stage.tile([P, 1], FP32, tag="rstd")
        nc.scalar.activation(rs, mv[:, 1:2], AF.Sqrt, bias=eps_t[:, 0:1])
        nc.vector.reciprocal(rs, rs)
        nc.vector.tensor_scalar(
            dst, res, mv[:, 0:1], rs, op0=ALU.subtract, op1=ALU.mult
        )
        nc.vector.tensor_mul(dst, dst, g_t)
        nc.vector.tensor_add(dst, dst, b_t)

    def open_ln_consts(gname, bname):
        lnp = pl.open("lnp", 1)
        g_t = lnp.tile([P, D], FP32, tag="ln_g")
        nc.sync.dma_start(g_t, _bcast_ap(ap(gname)))
        b_t = lnp.tile([P, D], FP32, tag="ln_b")
        nc.sync.dma_start(b_t, _bcast_ap(ap(bname)))
        return g_t, b_t

    def tap(src_big, cast_f32=False):
        for lt in range(NT):
            if cast_f32:
                o = stage.tile([P, D], FP32, tag="st32")
                nc.vector.tensor_copy(o, src_big[:, lt, :])
                nc.sync.dma_start(out_d.ap()[lt * P:(lt + 1) * P, :], o)
            else:
                nc.sync.dma_start(out_d.ap()[lt * P:(lt + 1) * P, :], src_big[:, lt, :])

    # ================= self attention =================
    project_qkv(xT, xT, "m_wq", "m_wk", "m_wv", "m_bq", "m_bk", "m_bv")
    pl.close("xT")
    sa_pool = pl.open("sa", 1, side="left")
    sa = sa_pool.tile([P, NT, D], BF16, tag="sa")
    attention(sa, causal=True)
    if stop_after == "sa":
        tap(sa, cast_f32=True)
        pl.close_all()
        return

    # ---- residual + LN1 -> x1_bf (bf16) and x1T (bf16) ----
    x1T_pool = pl.open("x1T", 1)
    x1T = x1T_pool.tile([P, DT, L], BF16, tag="x1T")
    g1, bb1 = open_ln_consts("ln1_g", "ln1_b")
    for lt in range(NT):
        res = stage.tile([P, D], FP32, tag="st32")
        nc.vector.tensor_add(res, x_bf[:, lt, :], sa[:, lt, :])
        lno = stage.tile([P, D], FP32, tag="st32")
        layer_norm(res, lno, g1, bb1)
        nc.vector.tensor_copy(x1_bf[:, lt, :], lno)
        transpose_into(x1T, x1_bf[:, lt, :], lt)
    pl.close("lnp")
    if stop_after == "x1":
        tap(x1_bf, cast_f32=True)
        pl.close_all()
        return
    pl.close("sa")
    pl.close("xres")

    # ================= cross attention =================
    project_qkv(x1T, encT, "c_wq", "c_wk", "c_wv", "c_bq", "c_bk", "c_bv")
    pl.close("x1T")
    ca_pool = pl.open("ca", 1, side="left")
    ca = ca_pool.tile([P, NT, D], BF16, tag="ca")
    attention(ca, causal=False)
    if stop_after == "ca":
        tap(ca, cast_f32=True)
        pl.close_all()
        return
    pl.close("qkv")
    pl.close("encT")

    # ---- residual + LN2 -> x2bf (bf16) and x2T (bf16) ----
    x2_pool = pl.open("x2", 1)
    x2bf = x2_pool.tile([P, NT, D], BF16, tag="x2bf")
    x2T = x2_pool.tile([P, DT, L], BF16, tag="x2T")
    g2, bb2 = open_ln_consts("ln2_g", "ln2_b")
    for lt in range(NT):
        res = stage.tile([P, D], FP32, tag="st32")
        nc.vector.tensor_add(res, x1_bf[:, lt, :], ca[:, lt, :])
        lno = stage.tile([P, D], FP32, tag="st32")
        layer_norm(res, lno, g2, bb2)
        nc.vector.tensor_copy(x2bf[:, lt, :], lno)
        transpose_into(x2T, x2bf[:, lt, :], lt)
    pl.close("lnp")
    if stop_after == "x2":
        tap(x2bf, cast_f32=True)
        pl.close_all()
        return
    pl.close("ca")
    pl.close("x1res")

    # ================= FFN =================
    ybp = pl.open("y_bf", 1, side="left")
    y_bf = ybp.tile([P, NT, D], BF16, tag="y_bf")
    ytp = pl.open("ybfT", 1, side="left")
    ybfT = ytp.tile([P, DT, L], BF16, tag="ybfT")
    for fh in range(FH):
        w1p = pl.open("w1p", 1)
        w1 = w1p.tile([P, DT, FT * P], BF16, tag="w1")
        for dt_ in range(DT):
            for half in range(2):
                c0 = fh * FT * P + half * (FT * P // 2)
                st = stage.tile([P, FT * P // 2], FP32, tag="st32")
                nc.sync.dma_start(
                    st, ap("ff_w1")[dt_ * P:(dt_ + 1) * P, c0:c0 + FT * P // 2]
                )
                nc.vector.tensor_copy(
                    w1[:, dt_, half * (FT * P // 2):(half + 1) * (FT * P // 2)], st
                )
        h1p = pl.open("h1p", 1)
        h1 = h1p.tile([P, FT, L], BF16, tag="h1")
        for ft in range(FT):
            gf = fh * FT + ft
            for lc in range(NLC):
                ps = psum.tile([P, LC], FP32, tag="ps")
                for dt_ in range(DT):
                    nc.tensor.matmul(
                        ps,
                        w1[:, dt_, ft * P:(ft + 1) * P],
                        x2T[:, dt_, lc * LC:(lc + 1) * LC],
                        start=(dt_ == 0),
                        stop=(dt_ == DT - 1),
                    )
                nc.scalar.activation(
                    h1[:, ft, lc * LC:(lc + 1) * LC],
                    ps,
                    AF.Relu,
                    bias=b1_col[:, gf:gf + 1],
                )
        w2p = pl.open("w2p", 1)
        w2 = w2p.tile([P, FT, D], BF16, tag="w2")
        for ft in range(FT):
            r0 = fh * FT * P + ft * P
            st = stage.tile([P, D], FP32, tag="st32")
            nc.sync.dma_start(st, ap("ff_w2")[r0:r0 + P, :])
            nc.vector.tensor_copy(w2[:, ft, :], st)
        for dc in range(DT):
            for lc in range(NLC):
                ps = psum.tile([P, LC], FP32, tag="ps")
                for ft in range(FT):
                    nc.tensor.matmul(
                        ps,
                        w2[:, ft, dc * P:(dc + 1) * P],
                        h1[:, ft, lc * LC:(lc + 1) * LC],
                        start=(ft == 0),
                        stop=(ft == FT - 1),
                    )
                sl = (slice(None), dc, slice(lc * LC, (lc + 1) * LC))
                if fh == 0:
                    nc.vector.tensor_scalar_add(ybfT[sl], ps, b2_col[:, dc:dc + 1])
                else:
                    nc.vector.tensor_add(ybfT[sl], ybfT[sl], ps)
        pl.close("w2p")
        pl.close("h1p")
        pl.close("w1p")

    for dc in range(DT):
        for lt in range(NT):
            nc.sync.dma_start(
                y_bf[:, lt, dc * P:(dc + 1) * P],
                ybfT[:, dc, lt * P:(lt + 1) * P],
                transpose=True,
            )
    pl.close("ybfT")
    if stop_after == "y":
        tap(y_bf, cast_f32=True)
        pl.close_all()
        return

    # ---- residual + LN3 (reuses ln2 params) -> out ----
    g3, bb3 = open_ln_consts("ln2_g", "ln2_b")
    for lt in range(NT):
        res = stage.tile([P, D], FP32, tag="st32")
        nc.vector.tensor_add(res, x2bf[:, lt, :], y_bf[:, lt, :])
        o = stage.tile([P, D], FP32, tag="st32")
        layer_norm(res, o, g3, bb3)
        nc.sync.dma_start(out_d.ap()[lt * P:(lt + 1) * P, :], o)

    pl.close_all()


_NC_CACHE = {}


def _get_nc(stop_after=None):
    key = stop_after
    if key not in _NC_CACHE:
        _NC_CACHE[key] = build(stop_after)
    return _NC_CACHE[key]


def kernel(**inputs):
    nc = _get_nc()
    xs = np.ascontiguousarray(np.asarray(inputs["decoder_embedding"], dtype=np.float32))
    es = np.ascontiguousarray(np.asarray(inputs["encoder_output"], dtype=np.float32))
    weights = {
        k: np.ascontiguousarray(np.asarray(inputs[k], dtype=np.float32))
        for k in WEIGHT_NAMES
    }
    in_maps = [
        {**weights, "decoder_embedding": xs[b], "encoder_output": es[b]}
        for b in range(B)
    ]
    res = run_bass_kernel_spmd(nc, in_maps, core_ids=list(range(B)))
    return np.stack([res.results[b]["out"] for b in range(B)], axis=0).astype(np.float32)



# revision 2
# speedup vs baseline: 6221.5874x; 6221.5874x over previous
"""Transformer decoder layer (masked self-attn + cross-attn + FFN, 3x LayerNorm)
for Trainium2, data-parallel over batch across 8 NeuronCores.

Per-core problem: L=1024 tokens, D=1024 model dim, H=16 heads x 64, DFF=4096.
Strategy: bf16 matmuls (fp32 PSUM accumulation), fp32 softmax/LN math.
Activations kept in [L, D] layout for LN/softmax; bf16 [D, L] transposed copies
(DMA xbar transpose) feed the TensorEngine. Attention computes S^T = K Q^T per
head ([Lk, Lq] layout), exp on ScalarE with the 1/sqrt(dk) scale folded in,
then O^T = [V | 1]^T expS^T which yields the softmax denominator as row 64 of
the PSUM output; O^T is DMA-transposed back and normalized with a native
per-partition scalar multiply. Causal masking skips fully-masked 128x512 score
blocks entirely and applies a precomputed 0/1 mask to diagonal blocks.

SBUF uses phase-scoped tile pools on the two LIFO stack sides:
left = residual carriers (x1_bf > x_bf > sa; ca; y), right = matmul operands
and weights (encT > qkv > per-phase pools). Peak ~200 KB/partition.
"""

import sys

sys.path.insert(0, "/opt/trn_rl_repo")

import numpy as np

import concourse.bass as bass
import concourse.mybir as mybir
import concourse.tile as tile
from concourse import bacc
from concourse.bass_utils import run_bass_kernel_spmd

FP32 = mybir.dt.float32
BF16 = mybir.dt.bfloat16
AF = mybir.ActivationFunctionType
ALU = mybir.AluOpType

B = 8
L = 1024
D = 1024
H = 16
DK = 64
DFF = 4096
P = 128
NT = L // P  # 8 l-tiles
DT = D // P  # 8 d-tiles
NP = H // 2  # 8 head pairs
LC = 512  # lq chunk
NLC = L // LC  # 2
FH = 2  # ffn dff halves
FT = DFF // FH // P  # 16 f-tiles per half
EPS = 1e-5

WEIGHT_NAMES = [
    "m_wq", "m_bq", "m_wk", "m_bk", "m_wv", "m_bv",
    "c_wq", "c_bq", "c_wk", "c_bk", "c_wv", "c_bv",
    "ff_w1", "ff_b1", "ff_w2", "ff_b2",
    "ln1_g", "ln1_b", "ln2_g", "ln2_b",
]

INPUT_SPECS = {
    "decoder_embedding": [L, D],
    "encoder_output": [L, D],
    "m_wq": [H, D, DK], "m_bq": [H, DK],
    "m_wk": [H, D, DK], "m_bk": [H, DK],
    "m_wv": [H, D, DK], "m_bv": [H, DK],
    "c_wq": [H, D, DK], "c_bq": [H, DK],
    "c_wk": [H, D, DK], "c_bk": [H, DK],
    "c_wv": [H, D, DK], "c_bv": [H, DK],
    "ff_w1": [D, DFF], "ff_b1": [DFF],
    "ff_w2": [DFF, D], "ff_b2": [D],
    "ln1_g": [D], "ln1_b": [D],
    "ln2_g": [D], "ln2_b": [D],
}


def _bcast_ap(ap, parts=P):
    """Broadcast a 1-D DRAM AP across `parts` partitions (step-0 partition dim)."""
    return bass.AP(tensor=ap.tensor, offset=ap.offset, ap=[[0, parts]] + list(ap.ap))


class Pools:
    """Manual pool open/close; per-side LIFO order is asserted at build time."""

    def __init__(self, tc):
        self.tc = tc
        self.stacks = {"left": [], "right": []}

    def open(self, name, bufs, side="right", space=bass.MemorySpace.SBUF):
        cm = self.tc.tile_pool(name=name, bufs=bufs, side=side, space=space)
        pool = cm.__enter__()
        self.stacks[side].append((name, cm))
        return pool

    def close(self, name):
        for side, stack in self.stacks.items():
            for i, (n, cm) in enumerate(stack):
                if n == name:
                    assert i == len(stack) - 1, (
                        f"pool {name} is not on top of {side} stack: "
                        f"{[x[0] for x in stack]}"
                    )
                    stack.pop()
                    cm.__exit__(None, None, None)
                    return
        raise KeyError(name)

    def close_all(self):
        for side in ("left", "right"):
            while self.stacks[side]:
                _, cm = self.stacks[side].pop()
                cm.__exit__(None, None, None)


def build(stop_after=None, n_bodies=1):
    nc = bacc.Bacc("TRN2", target_bir_lowering=False, debug=False, num_devices=B)

    dram = {}
    for name, shape in INPUT_SPECS.items():
        dram[name] = nc.dram_tensor(name, shape, FP32, kind="ExternalInput")
    out_d = nc.dram_tensor("out", [L, D], FP32, kind="ExternalOutput")

    with tile.TileContext(nc) as tc:
        for _ in range(n_bodies):
            _emit(nc, tc, dram, out_d, stop_after)
    nc.compile()
    return nc


def _emit(nc, tc, dram, out_d, stop_after):
    pl = Pools(tc)
    const = pl.open("const", 1)
    stage = pl.open("stage", 3)
    psum = pl.open("psum", 8, space=bass.MemorySpace.PSUM)

    def ap(name):
        return dram[name].ap()

    # ---- constants ----
    eps_t = const.tile([P, 1], FP32)
    nc.vector.memset(eps_t, EPS)

    # causal 0/1 masks for diagonal blocks: mask[i][kk, qq] = 1 if qq >= kk + i*128
    mask_bf = const.tile([P, 4, LC], BF16)
    for i in range(4):
        m32 = stage.tile([P, LC], FP32, tag="st32")
        nc.vector.memset(m32, 1.0)
        nc.gpsimd.affine_select(
            out=m32,
            in_=m32,
            compare_op=ALU.is_ge,
            fill=0.0,
            base=-(i * P),
            pattern=[[1, LC]],
            channel_multiplier=-1,
        )
        nc.vector.tensor_copy(mask_bf[:, i, :], m32)

    def load_bias_pair_col(name):
        # [H, DK] -> [128, NP]: partition = (h%2)*64 + k, col = h//2
        t = const.tile([P, NP], FP32, tag=f"bc_{name}")
        nc.sync.dma_start(t, ap(name).rearrange("(pr i) k -> (i k) pr", i=2))
        return t

    b1_col = const.tile([P, DFF // P], FP32)
    nc.sync.dma_start(b1_col, ap("ff_b1").rearrange("(ft p) -> p ft", p=P))
    b2_col = const.tile([P, DT], FP32)
    nc.sync.dma_start(b2_col, ap("ff_b2").rearrange("(dt p) -> p dt", p=P))

    # ---- long-lived tiles ----
    # LEFT: residual carriers; RIGHT: matmul operands
    x1res_pool = pl.open("x1res", 1, side="left")
    x1_bf = x1res_pool.tile([P, NT, D], BF16, tag="x1_bf")
    xres_pool = pl.open("xres", 1, side="left")
    x_bf = xres_pool.tile([P, NT, D], BF16, tag="x_bf")

    encT_pool = pl.open("encT", 1)
    encT = encT_pool.tile([P, DT, L], BF16, tag="encT")
    qkv = pl.open("qkv", 1)
    qt_t = qkv.tile([P, NP, L], BF16, tag="qt")
    kt_t = qkv.tile([P, NP, L], BF16, tag="kt")
    vn_t = qkv.tile([P, H, NT, 80], BF16, tag="vn")  # [.., 0:64]=V, 64=ones col
    xT_pool = pl.open("xT", 1)
    xT = xT_pool.tile([P, DT, L], BF16, tag="xT")

    def transpose_into(dstT, src_bf, lt):
        # src_bf [P, D] bf16 (l-tile lt) -> dstT[:, dt, lt*P:(lt+1)*P] per d-tile
        for dt_ in range(DT):
            nc.sync.dma_start(
                dstT[:, dt_, lt * P:(lt + 1) * P],
                src_bf[:, dt_ * P:(dt_ + 1) * P],
                transpose=True,
            )

    for lt in range(NT):
        e32 = stage.tile([P, D], FP32, tag="st32")
        nc.sync.dma_start(e32, ap("encoder_output")[lt * P:(lt + 1) * P, :])
        ebf = stage.tile([P, D], BF16, tag="stbf")
        nc.vector.tensor_copy(ebf, e32)
        transpose_into(encT, ebf, lt)

    for lt in range(NT):
        x32 = stage.tile([P, D], FP32, tag="st32")
        nc.sync.dma_start(x32, ap("decoder_embedding")[lt * P:(lt + 1) * P, :])
        nc.vector.tensor_copy(x_bf[:, lt, :], x32)
        transpose_into(xT, x_bf[:, lt, :], lt)

    # ---- helpers ----
    def load_attn_w(wpool, name):
        # [H, D, DK] -> [128(d), DT, H*DK] bf16; col = h*64+k = pair*128 + (h%2)*64 + k
        w = wpool.tile([P, DT, H * DK], BF16, tag="w_attn")
        for dt_ in range(DT):
            st = stage.tile([P, H, DK], FP32, tag="st32")
            nc.sync.dma_start(
                st, ap(name)[:, dt_ * P:(dt_ + 1) * P, :].rearrange("h d k -> d h k")
            )
            nc.vector.tensor_copy(
                w[:, dt_, :].rearrange("p (h k) -> p h k", h=H), st
            )
        return w

    def project(dst, w, b_col, srcT):
        # dst [P, NP, L] bf16: dst[i*64+k, pr, l] = sum_d srcT[d, l] w[d, pr*128+i*64+k] + b
        for pr in range(NP):
            for lc in range(NLC):
                ps = psum.tile([P, LC], FP32, tag="ps")
                for dt_ in range(DT):
                    nc.tensor.matmul(
                        ps,
                        w[:, dt_, pr * P:(pr + 1) * P],
                        srcT[:, dt_, lc * LC:(lc + 1) * LC],
                        start=(dt_ == 0),
                        stop=(dt_ == DT - 1),
                    )
                nc.vector.tensor_scalar_add(
                    dst[:, pr, lc * LC:(lc + 1) * LC], ps, b_col[:, pr:pr + 1]
                )

    def project_qkv(srcT_q, srcT_kv, wq_n, wk_n, wv_n, bq_n, bk_n, bv_n):
        wpool = pl.open("w_attn", 2)
        vt_pool = pl.open("vt", 1)
        vt = vt_pool.tile([P, NP, L], BF16, tag="vt")
        bq = load_bias_pair_col(bq_n)
        bk = load_bias_pair_col(bk_n)
        bv = load_bias_pair_col(bv_n)
        wq = load_attn_w(wpool, wq_n)
        project(qt_t, wq, bq, srcT_q)
        wk = load_attn_w(wpool, wk_n)
        project(kt_t, wk, bk, srcT_kv)
        wv = load_attn_w(wpool, wv_n)
        project(vt, wv, bv, srcT_kv)
        # V^T packed pairs -> vn [128(lk), H, NT, 80] with ones col 64
        nc.vector.memset(vn_t[:, :, :, 64:65], 1.0)
        for h in range(H):
            pr, i = divmod(h, 2)
            for kt_ in range(NT):
                nc.sync.dma_start(
                    vn_t[:, h, kt_, 0:64],
                    vt[i * 64:(i + 1) * 64, pr, kt_ * P:(kt_ + 1) * P],
                    transpose=True,
                )
        pl.close("vt")
        pl.close("w_attn")

    def attention(out_sa, causal):
        heads = pl.open("heads", 2)
        otrp = pl.open("otr", 3)
        for h in range(H):
            pr, i = divmod(h, 2)
            r0 = i * 64
            expS = heads.tile([P, NT, L], BF16, tag="expS")
            for lc in range(NLC):
                kts = range(4) if (causal and lc == 0) else range(NT)
                for kt_ in kts:
                    sps = psum.tile([P, LC], FP32, tag="ps")
                    nc.tensor.matmul(
                        sps,
                        kt_t[r0:r0 + 64, pr, kt_ * P:(kt_ + 1) * P],
                        qt_t[r0:r0 + 64, pr, lc * LC:(lc + 1) * LC],
                        start=True,
                        stop=True,
                        tile_position=(r0, 0),
                    )
                    dst = expS[:, kt_, lc * LC:(lc + 1) * LC]
                    nc.scalar.activation(dst, sps, AF.Exp, scale=0.125)
                    if causal and kt_ >= 4 * lc:
                        mi = kt_ - 4 * lc
                        nc.vector.tensor_mul(dst, dst, mask_bf[:, mi, :])
            # O^T (rows 0:64) + softmax denominator (row 64)
            ot = heads.tile([96, L], BF16, tag="ot")
            nc.vector.memset(ot[64:96, :], 0.0)
            for lc in range(NLC):
                kts = list(range(4)) if (causal and lc == 0) else list(range(NT))
                ops = psum.tile([P, LC], FP32, tag="ps")
                for j, kt_ in enumerate(kts):
                    nc.tensor.matmul(
                        ops[0:65, :],
                        vn_t[:, h, kt_, 0:65],
                        expS[:, kt_, lc * LC:(lc + 1) * LC],
                        start=(j == 0),
                        stop=(j == len(kts) - 1),
                    )
                nc.vector.tensor_copy(ot[0:65, lc * LC:(lc + 1) * LC], ops[0:65, :])
            otr = otrp.tile([P, NT, 96], BF16, tag="otr")
            for lt in range(NT):
                nc.sync.dma_start(
                    otr[:, lt, :], ot[:, lt * P:(lt + 1) * P], transpose=True
                )
            rcp = heads.tile([P, NT], FP32, tag="rcp")
            for lt in range(NT):
                nc.vector.reciprocal(rcp[:, lt:lt + 1], otr[:, lt, 64:65])
            for lt in range(NT):
                nc.vector.tensor_scalar_mul(
                    out_sa[:, lt, h * 64:(h + 1) * 64],
                    otr[:, lt, 0:64],
                    rcp[:, lt:lt + 1],
                )
        pl.close("otr")
        pl.close("heads")

    def layer_norm(res, dst, g_t, b_t):
        # res [P, D] f32 -> dst [P, D]
        st = stage.tile([P, 2, 6], FP32, tag="bnst")
        for c in range(2):
            nc.vector.bn_stats(st[:, c, :], res[:, c * 512:(c + 1) * 512])
        mv = stage.tile([P, 2], FP32, tag="bnmv")
        nc.vector.bn_aggr(mv, st)
        rs = stage.tile([P, 1], FP32, tag="rstd")
        nc.scalar.activation(rs, mv[:, 1:2], AF.Sqrt, bias=eps_t[:, 0:1])
        nc.vector.reciprocal(rs, rs)
        nc.vector.tensor_scalar(
            dst, res, mv[:, 0:1], rs, op0=ALU.subtract, op1=ALU.mult
        )
        nc.vector.tensor_mul(dst, dst, g_t)
        nc.vector.tensor_add(dst, dst, b_t)

    def open_ln_consts(gname, bname):
        lnp = pl.open("lnp", 1)
        g_t = lnp.tile([P, D], FP32, tag="ln_g")
        nc.sync.dma_start(g_t, _bcast_ap(ap(gname)))
        b_t = lnp.tile([P, D], FP32, tag="ln_b")
        nc.sync.dma_start(b_t, _bcast_ap(ap(bname)))
        return g_t, b_t

    def tap(src_big, cast_f32=False):
        for lt in range(NT):
            if cast_f32:
                o = stage.tile([P, D], FP32, tag="st32")
                nc.vector.tensor_copy(o, src_big[:, lt, :])
                nc.sync.dma_start(out_d.ap()[lt * P:(lt + 1) * P, :], o)
            else:
                nc.sync.dma_start(out_d.ap()[lt * P:(lt + 1) * P, :], src_big[:, lt, :])

    # ================= self attention =================
    project_qkv(xT, xT, "m_wq", "m_wk", "m_wv", "m_bq", "m_bk", "m_bv")
    pl.close("xT")
    sa_pool = pl.open("sa", 1, side="left")
    sa = sa_pool.tile([P, NT, D], BF16, tag="sa")
    attention(sa, causal=True)
    if stop_after == "sa":
        tap(sa, cast_f32=True)
        pl.close_all()
        return

    # ---- residual + LN1 -> x1_bf (bf16) and x1T (bf16) ----
    x1T_pool = pl.open("x1T", 1)
    x1T = x1T_pool.tile([P, DT, L], BF16, tag="x1T")
    g1, bb1 = open_ln_consts("ln1_g", "ln1_b")
    for lt in range(NT):
        res = stage.tile([P, D], FP32, tag="st32")
        nc.vector.tensor_add(res, x_bf[:, lt, :], sa[:, lt, :])
        lno = stage.tile([P, D], FP32, tag="st32")
        layer_norm(res, lno, g1, bb1)
        nc.vector.tensor_copy(x1_bf[:, lt, :], lno)
        transpose_into(x1T, x1_bf[:, lt, :], lt)
    pl.close("lnp")
    if stop_after == "x1":
        tap(x1_bf, cast_f32=True)
        pl.close_all()
        return
    pl.close("sa")
    pl.close("xres")

    # ================= cross attention =================
    project_qkv(x1T, encT, "c_wq", "c_wk", "c_wv", "c_bq", "c_bk", "c_bv")
    pl.close("x1T")
    ca_pool = pl.open("ca", 1, side="left")
    ca = ca_pool.tile([P, NT, D], BF16, tag="ca")
    attention(ca, causal=False)
    if stop_after == "ca":
        tap(ca, cast_f32=True)
        pl.close_all()
        return
    pl.close("qkv")
    pl.close("encT")

    # ---- residual + LN2 -> x2bf (bf16) and x2T (bf16) ----
    x2_pool = pl.open("x2", 1)
    x2bf = x2_pool.tile([P, NT, D], BF16, tag="x2bf")
    x2T = x2_pool.tile([P, DT, L], BF16, tag="x2T")
    g2, bb2 = open_ln_consts("ln2_g", "ln2_b")
    for lt in range(NT):
        res = stage.tile([P, D], FP32, tag="st32")
        nc.vector.tensor_add(res, x1_bf[:, lt, :], ca[:, lt, :])
        lno = stage.tile([P, D], FP32, tag="st32")
        layer_norm(res, lno, g2, bb2)
        nc.vector.tensor_copy(x2bf[:, lt, :], lno)
        transpose_into(x2T, x2bf[:, lt, :], lt)
    pl.close("lnp")
    if stop_after == "x2":
        tap(x2bf, cast_f32=True)
        pl.close_all()
        return
    pl.close("ca")
    pl.close("x1res")

    # ================= FFN =================
    ybp = pl.open("y_bf", 1, side="left")
    y_bf = ybp.tile([P, NT, D], BF16, tag="y_bf")
    ytp = pl.open("ybfT", 1, side="left")
    ybfT = ytp.tile([P, DT, L], BF16, tag="ybfT")
    for fh in range(FH):
        w1p = pl.open("w1p", 1)
        w1 = w1p.tile([P, DT, FT * P], BF16, tag="w1")
        for dt_ in range(DT):
            for half in range(2):
                c0 = fh * FT * P + half * (FT * P // 2)
                st = stage.tile([P, FT * P // 2], FP32, tag="st32")
                nc.sync.dma_start(
                    st, ap("ff_w1")[dt_ * P:(dt_ + 1) * P, c0:c0 + FT * P // 2]
                )
                nc.vector.tensor_copy(
                    w1[:, dt_, half * (FT * P // 2):(half + 1) * (FT * P // 2)], st
                )
        h1p = pl.open("h1p", 1)
        h1 = h1p.tile([P, FT, L], BF16, tag="h1")
        for ft in range(FT):
            gf = fh * FT + ft
            for lc in range(NLC):
                ps = psum.tile([P, LC], FP32, tag="ps")
                for dt_ in range(DT):
                    nc.tensor.matmul(
                        ps,
                        w1[:, dt_, ft * P:(ft + 1) * P],
                        x2T[:, dt_, lc * LC:(lc + 1) * LC],
                        start=(dt_ == 0),
                        stop=(dt_ == DT - 1),
                    )
                nc.scalar.activation(
                    h1[:, ft, lc * LC:(lc + 1) * LC],
                    ps,
                    AF.Relu,
                    bias=b1_col[:, gf:gf + 1],
                )
        w2p = pl.open("w2p", 1)
        w2 = w2p.tile([P, FT, D], BF16, tag="w2")
        for ft in range(FT):
            r0 = fh * FT * P + ft * P
            st = stage.tile([P, D], FP32, tag="st32")
            nc.sync.dma_start(st, ap("ff_w2")[r0:r0 + P, :])
            nc.vector.tensor_copy(w2[:, ft, :], st)
        for dc in range(DT):
            for lc in range(NLC):
                ps = psum.tile([P, LC], FP32, tag="ps")
                for ft in range(FT):
                    nc.tensor.matmul(
                        ps,
                        w2[:, ft, dc * P:(dc + 1) * P],
                        h1[:, ft, lc * LC:(lc + 1) * LC],
                        start=(ft == 0),
                        stop=(ft == FT - 1),
                    )
                sl = (slice(None), dc, slice(lc * LC, (lc + 1) * LC))
                if fh == 0:
                    nc.vector.tensor_scalar_add(ybfT[sl], ps, b2_col[:, dc:dc + 1])
                else:
                    nc.vector.tensor_add(ybfT[sl], ybfT[sl], ps)
        pl.close("w2p")
        pl.close("h1p")
        pl.close("w1p")

    for dc in range(DT):
        for lt in range(NT):
            nc.sync.dma_start(
                y_bf[:, lt, dc * P:(dc + 1) * P],
                ybfT[:, dc, lt * P:(lt + 1) * P],
                transpose=True,
            )
    pl.close("ybfT")
    if stop_after == "y":
        tap(y_bf, cast_f32=True)
        pl.close_all()
        return

    # ---- residual + LN3 (reuses ln2 params) -> out ----
    g3, bb3 = open_ln_consts("ln2_g", "ln2_b")
    for lt in range(NT):
        res = stage.tile([P, D], FP32, tag="st32")
        nc.vector.tensor_add(res, x2bf[:, lt, :], y_bf[:, lt, :])
        o = stage.tile([P, D], FP32, tag="st32")
        layer_norm(res, o, g3, bb3)
        nc.sync.dma_start(out_d.ap()[lt * P:(lt + 1) * P, :], o)

    pl.close_all()


_NC_CACHE = {}


def _get_nc(stop_after=None):
    key = stop_after
    if key not in _NC_CACHE:
        _NC_CACHE[key] = build(stop_after)
    return _NC_CACHE[key]


def _make_in_maps(inputs):
    xs = np.ascontiguousarray(np.asarray(inputs["decoder_embedding"], dtype=np.float32))
    es = np.ascontiguousarray(np.asarray(inputs["encoder_output"], dtype=np.float32))
    weights = {
        k: np.ascontiguousarray(np.asarray(inputs[k], dtype=np.float32))
        for k in WEIGHT_NAMES
    }
    return [
        {**weights, "decoder_embedding": xs[b], "encoder_output": es[b]}
        for b in range(B)
    ]


def _gather(res):
    return np.stack([res.results[b]["out"] for b in range(B)], axis=0).astype(np.float32)


def kernel(**inputs):
    nc = _get_nc()
    res = run_bass_kernel_spmd(nc, _make_in_maps(inputs), core_ids=list(range(B)))
    return _gather(res)



# revision 11
# speedup vs baseline: 13207.3233x; 2.1228x over previous
"""Transformer decoder layer (masked self-attn + cross-attn + FFN, 3x LayerNorm)
for Trainium2, data-parallel over batch across 8 NeuronCores.

Per-core problem: L=1024 tokens, D=1024 model dim, H=16 heads x 64, DFF=4096.

v2 design (prior baseline measured 1.79 ms/core on the NTFF profile):
- Host prepacks everything to bf16 in SBUF-ready layouts; the kernel DMAs
  weights/activations straight into place (no on-device fp32 staging/casts).
- Blocked DMA_TRANSPOSE: one instruction per [1024,1024] matrix (DRAM source)
  or per [128,1024] row-block (SBUF source) instead of per-128x128 tile.
- V is projected directly into [key-token, h*65] layout (stationary = x^T
  chunk, moving = wv) with a ones column per head, so the O^T matmul also
  emits the softmax denominator; V needs no transposes at all.
- S^T = K^T.T @ Q^T per head, two heads packed into the PE via row groups.
- exp on ScalarE over 2-bank PSUM groups; per-phase batched Rsqrt for the
  LayerNorms (one ACT instruction per LN phase -> no table-set thrash).
- FFN uses h1 chunks as the stationary operand so y lands untransposed.
- SBUF: one shared pool of 10 x ~16.6KB slots (tag "m") recycled across
  phases + small pools; PSUM: 2x[128,2,512] "sc" + 2x[128,2,512] "pj".
"""

import sys

sys.path.insert(0, "/opt/trn_rl_repo")

import numpy as np
import ml_dtypes

import concourse.bass as bass
import concourse.mybir as mybir
import concourse.tile as tile
from concourse import bacc
from concourse.bass_utils import run_bass_kernel_spmd

FP32 = mybir.dt.float32
BF16 = mybir.dt.bfloat16
AF = mybir.ActivationFunctionType
ALU = mybir.AluOpType

B = 8
L = 1024
D = 1024
H = 16
DK = 64
DFF = 4096
P = 128
NT = L // P  # 8 l-tiles
DT = D // P  # 8 d-tiles
NP = H // 2  # 8 head pairs
LC = 512
NLC = L // LC  # 2
FQ = 4  # ffn dff quarters
FT = DFF // FQ // P  # 8 f-tiles per quarter
EPS = 1e-5

NP_BF16 = ml_dtypes.bfloat16

INPUT_SPECS = {
    "x": ([L, D], BF16),
    "enc": ([L, D], BF16),
    "wq_m": ([D, D], BF16), "wk_m": ([D, D], BF16), "wv_m": ([D, D], BF16),
    "wq_c": ([D, D], BF16), "wk_c": ([D, D], BF16), "wv_c": ([D, D], BF16),
    "bqk_m": ([2, D], FP32), "bqk_c": ([2, D], FP32),
    "bv_m": ([D], BF16), "bv_c": ([D], BF16),
    "w1": ([D, DFF], BF16), "w2": ([DFF, D], BF16),
    "b1": ([DFF], FP32), "b2": ([D], BF16),
    "g1": ([D], BF16), "bb1": ([D], BF16),
    "g2": ([D], BF16), "bb2": ([D], BF16),
}


def _bcast_ap(ap, parts=P):
    """Broadcast a 1-D DRAM AP across `parts` partitions (step-0 partition dim)."""
    return bass.AP(tensor=ap.tensor, offset=ap.offset, ap=[[0, parts]] + list(ap.ap))


def build(stop_after=None):
    nc = bacc.Bacc("TRN2", target_bir_lowering=False, debug=False, num_devices=B)

    dram = {}
    for name, (shape, dt) in INPUT_SPECS.items():
        dram[name] = nc.dram_tensor(name, shape, dt, kind="ExternalInput")
    out_d = nc.dram_tensor("out", [L, D], FP32, kind="ExternalOutput")

    with tile.TileContext(nc) as tc:
        _emit(nc, tc, dram, out_d, stop_after)
    nc.compile()
    return nc


def _emit(nc, tc, dram, out_d, stop_after):
    with tc.tile_pool(name="const", bufs=1) as const, \
         tc.tile_pool(name="m", bufs=10) as m, \
         tc.tile_pool(name="heads", bufs=3) as heads, \
         tc.tile_pool(name="stage", bufs=2, side="left") as stage, \
         tc.tile_pool(name="pj", bufs=2, space=bass.MemorySpace.PSUM) as psum_pj, \
         tc.tile_pool(name="sc", bufs=2, space=bass.MemorySpace.PSUM) as psum_sc:
        _body(nc, dram, out_d, stop_after, const, m, heads, stage, psum_pj, psum_sc)


def _body(nc, dram, out_d, stop_after, const, m, heads, stage, psum_pj, psum_sc):
    def ap(name):
        return dram[name].ap()

    # ---- constants ----
    eps_t = const.tile([P, 1], FP32)
    nc.vector.memset(eps_t, EPS)

    # causal 0/1 masks for diagonal blocks: mask[i][kk, qq] = 1 if qq >= kk + i*128
    mask_bf = const.tile([P, 4, LC], BF16)
    for i in range(4):
        m32 = stage.tile([P, LC], FP32, tag="mst")
        nc.vector.memset(m32, 1.0)
        nc.gpsimd.affine_select(
            out=m32,
            in_=m32,
            compare_op=ALU.is_ge,
            fill=0.0,
            base=-(i * P),
            pattern=[[1, LC]],
            channel_multiplier=-1,
        )
        nc.vector.tensor_copy(mask_bf[:, i, :], m32)

    # per-partition bias columns for Q/K projections: [128, 2(j=q/k), 8(pr)]
    bqk = {}
    for name in ("bqk_m", "bqk_c"):
        t = const.tile([P, 2, NP], FP32, tag=name)
        nc.sync.dma_start(t, ap(name).rearrange("j (pr p) -> p j pr", p=P))
        bqk[name] = t
    # per-partition bias for FFN1 relu: [128, 32]
    b1_col = const.tile([P, DFF // P], FP32)
    nc.sync.dma_start(b1_col, ap("b1").rearrange("(ft p) -> p ft", p=P))
    # broadcast (all-partition) bias/param rows, bf16 [128, 1024]
    bcast = {}
    for name in ("bv_m", "bv_c", "b2", "g1", "bb1", "g2", "bb2"):
        t = const.tile([P, D], BF16, tag=f"bc_{name}")
        nc.sync.dma_start(t, _bcast_ap(ap(name)))
        bcast[name] = t

    # ---- inputs ----
    encT = m.tile([P, DT, L], BF16, tag="m")
    nc.sync.dma_start(encT, ap("enc"), transpose=True)
    xT = m.tile([P, DT, L], BF16, tag="m")
    nc.sync.dma_start(xT, ap("x"), transpose=True)

    # ---- helpers ----
    def load_w(name):
        w = m.tile([P, DT, D], BF16, tag="m")
        nc.scalar.dma_start(w, ap(name).rearrange("(dt p) c -> p dt c", p=P))
        return w

    def project_qk(wname, b_col, j, srcT):
        # returns [128(i*64+k), NP, L]: per head-pair column block of W^T srcT + b
        dst = m.tile([P, NP, L], BF16, tag="m")
        w = load_w(wname)
        for pr in range(NP):
            ps = psum_pj.tile([P, NLC, LC], FP32, tag="pj")
            for dt in range(DT):
                lhsT = w[:, dt, pr * P:(pr + 1) * P]
                for lc in range(NLC):
                    nc.tensor.matmul(
                        ps[:, lc, :],
                        lhsT,
                        srcT[:, dt, lc * LC:(lc + 1) * LC],
                        start=(dt == 0),
                        stop=(dt == DT - 1),
                    )
            nc.vector.tensor_scalar_add(
                dst[:, pr, :].rearrange("p (a b) -> p a b", a=NLC),
                ps,
                b_col[:, j, pr:pr + 1],
            )
        return dst

    def project_v(wname, bv_bc, srcT):
        # V [128(lk), NT, H*65]: V[:, kt, h*65+v] = (srcT_chunk.T @ wv)[lk, h*64+v] + bv
        # col 65*h+64 is a ones column.
        V = m.tile([P, NT, H * 65], BF16, tag="m")
        w = load_w(wname)
        for kt in range(NT):
            ps = psum_pj.tile([P, NLC, LC], FP32, tag="pj")
            for dt in range(DT):
                lhsT = srcT[:, dt, kt * P:(kt + 1) * P]
                for lc in range(NLC):
                    nc.tensor.matmul(
                        ps[:, lc, :],
                        lhsT,
                        w[:, dt, lc * LC:(lc + 1) * LC],
                        start=(dt == 0),
                        stop=(dt == DT - 1),
                    )
            Vv = V[:, kt, :].rearrange("p (h c) -> p h c", c=65)
            for lc in range(NLC):
                nc.vector.tensor_add(
                    Vv[:, lc * 8:(lc + 1) * 8, 0:64],
                    ps[:, lc, :].rearrange("p (h c) -> p h c", c=64),
                    bv_bc[:, lc * LC:(lc + 1) * LC].rearrange(
                        "p (h c) -> p h c", c=64
                    ),
                )
        nc.vector.memset(
            V.rearrange("p a (h c) -> p a h c", c=65)[:, :, :, 64:65], 1.0
        )
        return V

    def attention(out_sa, causal, qt, kt, V):
        for pr in range(NP):
            eS0 = m.tile([P, NT, L], BF16, tag="m")
            eS1 = m.tile([P, NT, L], BF16, tag="m")
            eS = [eS0, eS1]
            for lc in range(NLC):
                kts = list(range(4)) if (causal and lc == 0) else list(range(NT))
                for g0 in range(0, len(kts), 2):
                    grp = kts[g0:g0 + 2]
                    ps0 = psum_sc.tile([P, 2, LC], FP32, tag="sc")
                    ps1 = psum_sc.tile([P, 2, LC], FP32, tag="sc")
                    pss = [ps0, ps1]
                    for j, kt_ in enumerate(grp):
                        for i in range(2):
                            r0 = i * 64
                            nc.tensor.matmul(
                                pss[i][:, j, :],
                                kt[r0:r0 + 64, pr, kt_ * P:(kt_ + 1) * P],
                                qt[r0:r0 + 64, pr, lc * LC:(lc + 1) * LC],
                                start=True,
                                stop=True,
                                tile_position=(r0, 0),
                            )
                    for i in range(2):
                        nc.scalar.activation(
                            eS[i][:, grp[0]:grp[0] + len(grp),
                                  lc * LC:(lc + 1) * LC],
                            pss[i][:, 0:len(grp), :],
                            AF.Exp,
                            scale=0.125,
                        )
                    if causal:
                        for kt_ in grp:
                            if kt_ >= 4 * lc:
                                mi = kt_ - 4 * lc
                                for i in range(2):
                                    nc.vector.tensor_mul(
                                        eS[i][:, kt_, lc * LC:(lc + 1) * LC],
                                        eS[i][:, kt_, lc * LC:(lc + 1) * LC],
                                        mask_bf[:, mi, :],
                                    )
            # O^T rows 0:64 + softmax denominator row 64 (ones column of V)
            for i in range(2):
                h = 2 * pr + i
                av = psum_pj.tile([P, NLC, LC], FP32, tag="pj")
                for lc in range(NLC):
                    kts = list(range(4)) if (causal and lc == 0) else list(range(NT))
                    for j, kt_ in enumerate(kts):
                        nc.tensor.matmul(
                            av[0:65, lc, :],
                            V[:, kt_, h * 65:h * 65 + 65],
                            eS[i][:, kt_, lc * LC:(lc + 1) * LC],
                            start=(j == 0),
                            stop=(j == len(kts) - 1),
                        )
                ot = heads.tile([80, L], BF16, tag="ot")
                nc.vector.memset(ot[64:80, :], 0.0)
                nc.vector.tensor_copy(
                    ot[0:65, :].rearrange("p (a b) -> p a b", a=NLC), av[0:65]
                )
                otr = heads.tile([P, NT, 80], BF16, tag="otr")
                nc.sync.dma_start(otr, ot, transpose=True)
                rcp = heads.tile([P, NT, 1], FP32, tag="rcp")
                nc.vector.reciprocal(rcp, otr[:, :, 64:65])
                nc.vector.tensor_mul(
                    out_sa.rearrange("p lt (hh c) -> p lt hh c", c=64)[:, :, h, :],
                    otr[:, :, 0:64],
                    rcp.broadcast_to([P, NT, 64]),
                )

    def ln_phase(a_big, b_big, g_t, b_t, emit_block, res_name="res"):
        # residual r = a+b per block; batched stats -> one Rsqrt; then
        # emit_block(lt, z_fn) where z_fn(dst_dtype_tile_tag) builds the
        # normalized+affine output for block lt.
        res = m.tile([P, NT, D], BF16, tag="m")
        mvall = stage.tile([P, NT, 2], FP32, tag="mv")
        for lt in range(NT):
            nc.vector.tensor_add(res[:, lt, :], a_big[:, lt, :], b_big[:, lt, :])
            st = stage.tile([P, 2, 6], FP32, tag="bnst")
            nc.vector.bn_stats(st[:, 0, :], res[:, lt, 0:512])
            nc.vector.bn_stats(st[:, 1, :], res[:, lt, 512:1024])
            nc.vector.bn_aggr(mvall[:, lt, :], st)
        sq = stage.tile([P, NT, 1], FP32, tag="sq")
        nc.scalar.activation(sq, mvall[:, :, 1:2], AF.Sqrt, bias=eps_t[:, 0:1])
        rsq = stage.tile([P, NT, 1], FP32, tag="rsq")
        nc.vector.reciprocal(rsq, sq)
        mrs = stage.tile([P, NT, 1], FP32, tag="mrs")
        nc.vector.tensor_mul(mrs, mvall[:, :, 0:1], rsq)
        for lt in range(NT):
            emit_block(lt, res, rsq, mrs)
        return res

    def ln_finish(dst, res_lt, rsq_lt, mrs_lt, g_t, b_t, via=None):
        z = via if via is not None else dst
        nc.vector.tensor_scalar(
            z, res_lt, rsq_lt, mrs_lt, op0=ALU.mult, op1=ALU.subtract
        )
        nc.vector.tensor_mul(dst, z, g_t)
        nc.vector.tensor_add(dst, dst, b_t)

    def tap(src_big):
        for lt in range(NT):
            o = stage.tile([P, D], FP32, tag="zf")
            nc.vector.tensor_copy(o, src_big[:, lt, :])
            nc.sync.dma_start(out_d.ap()[lt * P:(lt + 1) * P, :], o)

    # ================= self attention =================
    qt_s = project_qk("wq_m", bqk["bqk_m"], 0, xT)
    kt_s = project_qk("wk_m", bqk["bqk_m"], 1, xT)
    V_s = project_v("wv_m", bcast["bv_m"], xT)
    # xT's slot is recycled after V_s projection (last reader)

    sa = m.tile([P, NT, D], BF16, tag="m")
    attention(sa, True, qt_s, kt_s, V_s)
    if stop_after == "sa":
        tap(sa)
        return

    # cross K/V projections (can fill PE gaps at the tail of self-attn)
    kt_c = project_qk("wk_c", bqk["bqk_c"], 1, encT)
    V_c = project_v("wv_c", bcast["bv_c"], encT)

    # ---- residual + LN1 -> x1 (bf16) and x1T ----
    x_res = m.tile([P, NT, D], BF16, tag="m")
    nc.sync.dma_start(x_res, ap("x").rearrange("(lt p) d -> p lt d", p=P))
    x1 = m.tile([P, NT, D], BF16, tag="m")
    x1T = m.tile([P, DT, L], BF16, tag="m")

    def emit_ln1(lt, res, rsq, mrs):
        z = stage.tile([P, D], BF16, tag="zb")
        ln_finish(x1[:, lt, :], res[:, lt, :], rsq[:, lt, :], mrs[:, lt, :],
                  bcast["g1"], bcast["bb1"], via=z)
        nc.sync.dma_start(
            x1T[:, :, lt * P:(lt + 1) * P], x1[:, lt, :], transpose=True
        )

    ln_phase(x_res, sa, bcast["g1"], bcast["bb1"], emit_ln1)
    if stop_after == "x1":
        tap(x1)
        return

    # ================= cross attention =================
    qt_c = project_qk("wq_c", bqk["bqk_c"], 0, x1T)
    ca = m.tile([P, NT, D], BF16, tag="m")
    attention(ca, False, qt_c, kt_c, V_c)
    if stop_after == "ca":
        tap(ca)
        return

    # ---- residual + LN2 -> x2 (bf16) and x2T ----
    x2 = m.tile([P, NT, D], BF16, tag="m")
    x2T = m.tile([P, DT, L], BF16, tag="m")

    def emit_ln2(lt, res, rsq, mrs):
        z = stage.tile([P, D], BF16, tag="zb")
        ln_finish(x2[:, lt, :], res[:, lt, :], rsq[:, lt, :], mrs[:, lt, :],
                  bcast["g2"], bcast["bb2"], via=z)
        nc.sync.dma_start(
            x2T[:, :, lt * P:(lt + 1) * P], x2[:, lt, :], transpose=True
        )

    ln_phase(x1, ca, bcast["g2"], bcast["bb2"], emit_ln2)
    if stop_after == "x2":
        tap(x2)
        return

    # ================= FFN (dff quarters) =================
    y_bf = m.tile([P, NT, D], BF16, tag="m")
    res3 = None
    for q in range(FQ):
        w1 = m.tile([P, DT, FT * P], BF16, tag="m")
        nc.scalar.dma_start(
            w1,
            ap("w1")[:, q * FT * P:(q + 1) * FT * P].rearrange(
                "(dt p) c -> p dt c", p=P
            ),
        )
        h1 = m.tile([P, FT, L], BF16, tag="m")
        for ft in range(FT):
            ps = psum_sc.tile([P, NLC, LC], FP32, tag="sc")
            for dt in range(DT):
                lhsT = w1[:, dt, ft * P:(ft + 1) * P]
                for lc in range(NLC):
                    nc.tensor.matmul(
                        ps[:, lc, :],
                        lhsT,
                        x2T[:, dt, lc * LC:(lc + 1) * LC],
                        start=(dt == 0),
                        stop=(dt == DT - 1),
                    )
            nc.scalar.activation(
                h1[:, ft, :].rearrange("p (a b) -> p a b", a=NLC),
                ps,
                AF.Relu,
                bias=b1_col[:, q * FT + ft:q * FT + ft + 1],
            )
        w2 = m.tile([P, FT, D], BF16, tag="m")
        nc.scalar.dma_start(
            w2,
            ap("w2")[q * FT * P:(q + 1) * FT * P, :].rearrange(
                "(ft p) c -> p ft c", p=P
            ),
        )
        if q == FQ - 1:
            res3 = m.tile([P, NT, D], BF16, tag="m")
        for lb in range(NT):
            ps = psum_pj.tile([P, NLC, LC], FP32, tag="pj")
            for ft in range(FT):
                lhsT = h1[:, ft, lb * P:(lb + 1) * P]
                for lc in range(NLC):
                    nc.tensor.matmul(
                        ps[:, lc, :],
                        lhsT,
                        w2[:, ft, lc * LC:(lc + 1) * LC],
                        start=(ft == 0),
                        stop=(ft == FT - 1),
                    )
            psv = ps.rearrange("p a b -> p (a b)")
            if q == 0:
                nc.vector.tensor_add(y_bf[:, lb, :], psv, bcast["b2"])
            elif q < FQ - 1:
                nc.vector.tensor_add(y_bf[:, lb, :], y_bf[:, lb, :], psv)
            else:
                t = stage.tile([P, D], BF16, tag="zb")
                nc.vector.tensor_add(t, y_bf[:, lb, :], psv)
                nc.vector.tensor_add(res3[:, lb, :], t, x2[:, lb, :])

    # ---- final LN (reuses ln2 params), fp32 out ----
    mvall = stage.tile([P, NT, 2], FP32, tag="mv")
    for lt in range(NT):
        st = stage.tile([P, 2, 6], FP32, tag="bnst")
        nc.vector.bn_stats(st[:, 0, :], res3[:, lt, 0:512])
        nc.vector.bn_stats(st[:, 1, :], res3[:, lt, 512:1024])
        nc.vector.bn_aggr(mvall[:, lt, :], st)
    sq = stage.tile([P, NT, 1], FP32, tag="sq")
    nc.scalar.activation(sq, mvall[:, :, 1:2], AF.Sqrt, bias=eps_t[:, 0:1])
    rsq = stage.tile([P, NT, 1], FP32, tag="rsq")
    nc.vector.reciprocal(rsq, sq)
    mrs = stage.tile([P, NT, 1], FP32, tag="mrs")
    nc.vector.tensor_mul(mrs, mvall[:, :, 0:1], rsq)
    for lt in range(NT):
        o = stage.tile([P, D], FP32, tag="zf")
        ln_finish(o, res3[:, lt, :], rsq[:, lt, :], mrs[:, lt, :],
                  bcast["g2"], bcast["bb2"])
        nc.sync.dma_start(out_d.ap()[lt * P:(lt + 1) * P, :], o)


_NC_CACHE = {}


def _get_nc(stop_after=None):
    key = stop_after
    if key not in _NC_CACHE:
        _NC_CACHE[key] = build(stop_after)
    return _NC_CACHE[key]


def _pack_weights(inputs):
    """Host-side prepack: cast to bf16 and lay out as the kernel expects."""
    f32 = lambda k: np.ascontiguousarray(np.asarray(inputs[k], dtype=np.float32))
    bf = lambda a: np.ascontiguousarray(np.asarray(a, dtype=NP_BF16))

    def attn_w(k):
        # [H, D, DK] -> [D, H*DK] bf16
        w = f32(k).transpose(1, 0, 2).reshape(D, H * DK)
        return bf(w)

    return {
        "wq_m": attn_w("m_wq"), "wk_m": attn_w("m_wk"), "wv_m": attn_w("m_wv"),
        "wq_c": attn_w("c_wq"), "wk_c": attn_w("c_wk"), "wv_c": attn_w("c_wv"),
        "bqk_m": np.ascontiguousarray(
            np.stack([f32("m_bq").reshape(-1), f32("m_bk").reshape(-1)])
        ),
        "bqk_c": np.ascontiguousarray(
            np.stack([f32("c_bq").reshape(-1), f32("c_bk").reshape(-1)])
        ),
        "bv_m": bf(f32("m_bv").reshape(-1)),
        "bv_c": bf(f32("c_bv").reshape(-1)),
        "w1": bf(f32("ff_w1")),
        "w2": bf(f32("ff_w2")),
        "b1": f32("ff_b1"),
        "b2": bf(f32("ff_b2")),
        "g1": bf(f32("ln1_g")), "bb1": bf(f32("ln1_b")),
        "g2": bf(f32("ln2_g")), "bb2": bf(f32("ln2_b")),
    }


def _make_in_maps(inputs):
    xs = np.ascontiguousarray(
        np.asarray(inputs["decoder_embedding"], dtype=np.float32).astype(NP_BF16)
    )
    es = np.ascontiguousarray(
        np.asarray(inputs["encoder_output"], dtype=np.float32).astype(NP_BF16)
    )
    packed = _pack_weights(inputs)
    return [{**packed, "x": xs[b], "enc": es[b]} for b in range(B)]


def _gather(res):
    return np.stack([res.results[b]["out"] for b in range(B)], axis=0).astype(np.float32)


def kernel(**inputs):
    nc = _get_nc()
    res = run_bass_kernel_spmd(nc, _make_in_maps(inputs), core_ids=list(range(B)))
    return _gather(res)


# revision 19
# speedup vs baseline: 13428.8286x; 1.0168x over previous
"""Transformer decoder layer (masked self-attn + cross-attn + FFN, 3x LayerNorm)
for Trainium2, data-parallel over batch across 8 NeuronCores.

Per-core problem: L=1024 tokens, D=1024 model dim, H=16 heads x 64, DFF=4096.

v2 design (prior baseline measured 1.79 ms/core on the NTFF profile):
- Host prepacks everything to bf16 in SBUF-ready layouts; the kernel DMAs
  weights/activations straight into place (no on-device fp32 staging/casts).
- Blocked DMA_TRANSPOSE: one instruction per [1024,1024] matrix (DRAM source)
  or per [128,1024] row-block (SBUF source) instead of per-128x128 tile.
- V is projected directly into [key-token, h*65] layout (stationary = x^T
  chunk, moving = wv) with a ones column per head, so the O^T matmul also
  emits the softmax denominator; V needs no transposes at all.
- S^T = K^T.T @ Q^T per head, two heads packed into the PE via row groups.
- exp on ScalarE over 2-bank PSUM groups; per-phase batched Rsqrt for the
  LayerNorms (one ACT instruction per LN phase -> no table-set thrash).
- FFN uses h1 chunks as the stationary operand so y lands untransposed.
- SBUF: one shared pool of 10 x ~16.6KB slots (tag "m") recycled across
  phases + small pools; PSUM: 2x[128,2,512] "sc" + 2x[128,2,512] "pj".
"""

import sys

sys.path.insert(0, "/opt/trn_rl_repo")

import numpy as np
import ml_dtypes

import concourse.bass as bass
import concourse.mybir as mybir
import concourse.tile as tile
from concourse import bacc
from concourse.bass_utils import run_bass_kernel_spmd

FP32 = mybir.dt.float32
BF16 = mybir.dt.bfloat16
AF = mybir.ActivationFunctionType
ALU = mybir.AluOpType

B = 8
L = 1024
D = 1024
H = 16
DK = 64
DFF = 4096
P = 128
NT = L // P  # 8 l-tiles
DT = D // P  # 8 d-tiles
NP = H // 2  # 8 head pairs
LC = 512
NLC = L // LC  # 2
FQ = 4  # ffn dff quarters
FT = DFF // FQ // P  # 8 f-tiles per quarter
EPS = 1e-5

NP_BF16 = ml_dtypes.bfloat16

INPUT_SPECS = {
    "x": ([L, D], BF16),
    "enc": ([L, D], BF16),
    "wq_m": ([D, D], BF16), "wk_m": ([D, D], BF16), "wv_m": ([D, D], BF16),
    "wq_c": ([D, D], BF16), "wk_c": ([D, D], BF16), "wv_c": ([D, D], BF16),
    "bqk_m": ([2, D], FP32), "bqk_c": ([2, D], FP32),
    "bv_m": ([D], BF16), "bv_c": ([D], BF16),
    "w1": ([D, DFF], BF16), "w2": ([DFF, D], BF16),
    "b1": ([DFF], FP32), "b2": ([D], BF16),
    "g1": ([D], BF16), "bb1": ([D], BF16),
    "g2": ([D], BF16), "bb2": ([D], BF16),
}


def _bcast_ap(ap, parts=P):
    """Broadcast a 1-D DRAM AP across `parts` partitions (step-0 partition dim)."""
    return bass.AP(tensor=ap.tensor, offset=ap.offset, ap=[[0, parts]] + list(ap.ap))


def build(stop_after=None):
    nc = bacc.Bacc("TRN2", target_bir_lowering=False, debug=False, num_devices=B)

    dram = {}
    for name, (shape, dt) in INPUT_SPECS.items():
        dram[name] = nc.dram_tensor(name, shape, dt, kind="ExternalInput")
    out_d = nc.dram_tensor("out", [L, D], FP32, kind="ExternalOutput")

    with tile.TileContext(nc) as tc:
        _emit(nc, tc, dram, out_d, stop_after)
    nc.compile()
    return nc


def _emit(nc, tc, dram, out_d, stop_after):
    with tc.tile_pool(name="const", bufs=1) as const, \
         tc.tile_pool(name="m", bufs=10) as m, \
         tc.tile_pool(name="heads", bufs=3) as heads, \
         tc.tile_pool(name="stage", bufs=2, side="left") as stage, \
         tc.tile_pool(name="pj", bufs=2, space=bass.MemorySpace.PSUM) as psum_pj, \
         tc.tile_pool(name="sc", bufs=2, space=bass.MemorySpace.PSUM) as psum_sc:
        _body(nc, dram, out_d, stop_after, const, m, heads, stage, psum_pj, psum_sc)


def _body(nc, dram, out_d, stop_after, const, m, heads, stage, psum_pj, psum_sc):
    def ap(name):
        return dram[name].ap()

    # ---- constants ----
    eps_t = const.tile([P, 1], FP32)
    nc.vector.memset(eps_t, EPS)

    # causal 0/1 masks for diagonal blocks: mask[i][kk, qq] = 1 if qq >= kk + i*128
    mask_bf = const.tile([P, 4, LC], BF16)
    for i in range(4):
        m32 = stage.tile([P, LC], FP32, tag="mst")
        nc.vector.memset(m32, 1.0)
        nc.gpsimd.affine_select(
            out=m32,
            in_=m32,
            compare_op=ALU.is_ge,
            fill=0.0,
            base=-(i * P),
            pattern=[[1, LC]],
            channel_multiplier=-1,
        )
        nc.vector.tensor_copy(mask_bf[:, i, :], m32)

    # ---- inputs (xT first: the first projection waits on it) ----
    xT = m.tile([P, DT, L], BF16, tag="m")
    for lh in range(2):
        nc.sync.dma_start(
            xT[:, :, lh * LC:(lh + 1) * LC],
            ap("x")[lh * LC:(lh + 1) * LC, :],
            transpose=True,
        )
    encT = m.tile([P, DT, L], BF16, tag="m")
    nc.sync.dma_start(encT, ap("enc"), transpose=True)

    # per-partition bias columns for Q/K projections: [128, 2(j=q/k), 8(pr)]
    bqk = {}
    for name in ("bqk_m", "bqk_c"):
        t = const.tile([P, 2, NP], FP32, tag=name)
        nc.sync.dma_start(t, ap(name).rearrange("j (pr p) -> p j pr", p=P))
        bqk[name] = t
    # per-partition bias for FFN1 relu: [128, 32]
    b1_col = const.tile([P, DFF // P], FP32)
    nc.sync.dma_start(b1_col, ap("b1").rearrange("(ft p) -> p ft", p=P))
    # broadcast (all-partition) bias/param rows, bf16 [128, 1024]
    bcast = {}
    for name in ("bv_m", "bv_c", "b2", "g1", "bb1", "g2", "bb2"):
        t = const.tile([P, D], BF16, tag=f"bc_{name}")
        nc.sync.dma_start(t, _bcast_ap(ap(name)))
        bcast[name] = t

    # ---- helpers ----
    def load_w(name):
        w = m.tile([P, DT, D], BF16, tag="m")
        nc.scalar.dma_start(w, ap(name).rearrange("(dt p) c -> p dt c", p=P))
        return w

    def project_qk(wname, b_col, j, srcT, lc_outer=False):
        # returns [128(i*64+k), NP, L]: per head-pair column block of W^T srcT + b
        # lc_outer: emit all head-pairs for l-chunk 0 first so consumers of the
        # first chunk (and producers of only the first srcT l-columns) pipeline.
        dst = m.tile([P, NP, L], BF16, tag="m")
        w = load_w(wname)
        if lc_outer:
            for lc in range(NLC):
                for pr in range(NP):
                    ps = psum_pj.tile([P, 1, LC], FP32, tag="pj")
                    for dt in range(DT):
                        nc.tensor.matmul(
                            ps[:, 0, :],
                            w[:, dt, pr * P:(pr + 1) * P],
                            srcT[:, dt, lc * LC:(lc + 1) * LC],
                            start=(dt == 0),
                            stop=(dt == DT - 1),
                        )
                    nc.vector.tensor_scalar_add(
                        dst[:, pr, lc * LC:(lc + 1) * LC],
                        ps[:, 0, :],
                        b_col[:, j, pr:pr + 1],
                    )
            return dst
        for pr in range(NP):
            ps = psum_pj.tile([P, NLC, LC], FP32, tag="pj")
            for dt in range(DT):
                lhsT = w[:, dt, pr * P:(pr + 1) * P]
                for lc in range(NLC):
                    nc.tensor.matmul(
                        ps[:, lc, :],
                        lhsT,
                        srcT[:, dt, lc * LC:(lc + 1) * LC],
                        start=(dt == 0),
                        stop=(dt == DT - 1),
                    )
            nc.vector.tensor_scalar_add(
                dst[:, pr, :].rearrange("p (a b) -> p a b", a=NLC),
                ps,
                b_col[:, j, pr:pr + 1],
            )
        return dst

    def project_v(wname, bv_bc, srcT):
        # V [128(lk), NT, H*65]: V[:, kt, h*65+v] = (srcT_chunk.T @ wv)[lk, h*64+v] + bv
        # col 65*h+64 is a ones column.
        V = m.tile([P, NT, H * 65], BF16, tag="m")
        w = load_w(wname)
        for kt in range(NT):
            ps = psum_pj.tile([P, NLC, LC], FP32, tag="pj")
            for dt in range(DT):
                lhsT = srcT[:, dt, kt * P:(kt + 1) * P]
                for lc in range(NLC):
                    nc.tensor.matmul(
                        ps[:, lc, :],
                        lhsT,
                        w[:, dt, lc * LC:(lc + 1) * LC],
                        start=(dt == 0),
                        stop=(dt == DT - 1),
                    )
            Vv = V[:, kt, :].rearrange("p (h c) -> p h c", c=65)
            for lc in range(NLC):
                nc.vector.tensor_add(
                    Vv[:, lc * 8:(lc + 1) * 8, 0:64],
                    ps[:, lc, :].rearrange("p (h c) -> p h c", c=64),
                    bv_bc[:, lc * LC:(lc + 1) * LC].rearrange(
                        "p (h c) -> p h c", c=64
                    ),
                )
        nc.vector.memset(
            V.rearrange("p a (h c) -> p a h c", c=65)[:, :, :, 64:65], 1.0
        )
        return V

    def attention(out_sa, causal, qt, kt, V):
        for pr in range(NP):
            eS0 = m.tile([P, NT, L], BF16, tag="m")
            eS1 = m.tile([P, NT, L], BF16, tag="m")
            eS = [eS0, eS1]
            for lc in range(NLC):
                kts = list(range(4)) if (causal and lc == 0) else list(range(NT))
                for g0 in range(0, len(kts), 2):
                    grp = kts[g0:g0 + 2]
                    ps0 = psum_sc.tile([P, 2, LC], FP32, tag="sc")
                    ps1 = psum_sc.tile([P, 2, LC], FP32, tag="sc")
                    pss = [ps0, ps1]
                    for j, kt_ in enumerate(grp):
                        for i in range(2):
                            r0 = i * 64
                            nc.tensor.matmul(
                                pss[i][:, j, :],
                                kt[r0:r0 + 64, pr, kt_ * P:(kt_ + 1) * P],
                                qt[r0:r0 + 64, pr, lc * LC:(lc + 1) * LC],
                                start=True,
                                stop=True,
                                tile_position=(r0, 0),
                            )
                    for i in range(2):
                        nc.scalar.activation(
                            eS[i][:, grp[0]:grp[0] + len(grp),
                                  lc * LC:(lc + 1) * LC],
                            pss[i][:, 0:len(grp), :],
                            AF.Exp,
                            scale=0.125,
                        )
                    if causal:
                        for kt_ in grp:
                            if kt_ >= 4 * lc:
                                mi = kt_ - 4 * lc
                                for i in range(2):
                                    nc.vector.tensor_mul(
                                        eS[i][:, kt_, lc * LC:(lc + 1) * LC],
                                        eS[i][:, kt_, lc * LC:(lc + 1) * LC],
                                        mask_bf[:, mi, :],
                                    )
            # O^T rows 0:64 + softmax denominator row 64 (ones column of V)
            for i in range(2):
                h = 2 * pr + i
                av = psum_pj.tile([P, NLC, LC], FP32, tag="pj")
                for lc in range(NLC):
                    kts = list(range(4)) if (causal and lc == 0) else list(range(NT))
                    for j, kt_ in enumerate(kts):
                        nc.tensor.matmul(
                            av[0:65, lc, :],
                            V[:, kt_, h * 65:h * 65 + 65],
                            eS[i][:, kt_, lc * LC:(lc + 1) * LC],
                            start=(j == 0),
                            stop=(j == len(kts) - 1),
                        )
                ot = heads.tile([80, L], BF16, tag="ot")
                nc.vector.memset(ot[64:80, :], 0.0)
                nc.vector.tensor_copy(
                    ot[0:65, :].rearrange("p (a b) -> p a b", a=NLC), av[0:65]
                )
                otr = heads.tile([P, NT, 80], BF16, tag="otr")
                nc.sync.dma_start(otr, ot, transpose=True)
                rcp = heads.tile([P, NT, 1], FP32, tag="rcp")
                nc.vector.reciprocal(rcp, otr[:, :, 64:65])
                nc.vector.tensor_mul(
                    out_sa.rearrange("p lt (hh c) -> p lt hh c", c=64)[:, :, h, :],
                    otr[:, :, 0:64],
                    rcp.broadcast_to([P, NT, 64]),
                )

    def ln_phase(a_big, b_big, g_t, b_t, emit_block, res_name="res"):
        # residual r = a+b per block; stats batched per half (4 blocks) so the
        # first half's normalize + downstream work starts early.
        res = m.tile([P, NT, D], BF16, tag="m")
        mvall = stage.tile([P, NT, 2], FP32, tag="mv")
        rsq = stage.tile([P, NT, 1], FP32, tag="rsq")
        mrs = stage.tile([P, NT, 1], FP32, tag="mrs")
        for hf in range(2):
            lts = range(hf * 4, hf * 4 + 4)
            for lt in lts:
                nc.vector.tensor_add(res[:, lt, :], a_big[:, lt, :], b_big[:, lt, :])
                st = stage.tile([P, 2, 6], FP32, tag="bnst")
                nc.vector.bn_stats(st[:, 0, :], res[:, lt, 0:512])
                nc.vector.bn_stats(st[:, 1, :], res[:, lt, 512:1024])
                nc.vector.bn_aggr(mvall[:, lt, :], st)
            sq = stage.tile([P, 4, 1], FP32, tag="sq")
            nc.scalar.activation(
                sq, mvall[:, hf * 4:hf * 4 + 4, 1:2], AF.Sqrt, bias=eps_t[:, 0:1]
            )
            nc.vector.reciprocal(rsq[:, hf * 4:hf * 4 + 4, :], sq)
            nc.vector.tensor_mul(
                mrs[:, hf * 4:hf * 4 + 4, :],
                mvall[:, hf * 4:hf * 4 + 4, 0:1],
                rsq[:, hf * 4:hf * 4 + 4, :],
            )
            for lt in lts:
                emit_block(lt, res, rsq, mrs)
        return res

    def ln_finish(dst, res_lt, rsq_lt, mrs_lt, g_t, b_t, via=None):
        z = via if via is not None else dst
        nc.vector.tensor_scalar(
            z, res_lt, rsq_lt, mrs_lt, op0=ALU.mult, op1=ALU.subtract
        )
        nc.vector.tensor_mul(dst, z, g_t)
        nc.vector.tensor_add(dst, dst, b_t)

    def tap(src_big):
        for lt in range(NT):
            o = stage.tile([P, D], FP32, tag="zf")
            nc.vector.tensor_copy(o, src_big[:, lt, :])
            nc.sync.dma_start(out_d.ap()[lt * P:(lt + 1) * P, :], o)

    # ================= self attention =================
    qt_s = project_qk("wq_m", bqk["bqk_m"], 0, xT)
    kt_s = project_qk("wk_m", bqk["bqk_m"], 1, xT)
    V_s = project_v("wv_m", bcast["bv_m"], xT)
    # xT's slot is recycled after V_s projection (last reader)

    sa = m.tile([P, NT, D], BF16, tag="m")
    attention(sa, True, qt_s, kt_s, V_s)
    if stop_after == "sa":
        tap(sa)
        return

    # cross K/V projections (can fill PE gaps at the tail of self-attn)
    x_res = m.tile([P, NT, D], BF16, tag="m")
    nc.scalar.dma_start(x_res, ap("x").rearrange("(lt p) d -> p lt d", p=P))
    kt_c = project_qk("wk_c", bqk["bqk_c"], 1, encT)
    V_c = project_v("wv_c", bcast["bv_c"], encT)

    # ---- residual + LN1 -> x1 (bf16) and x1T ----
    x1 = m.tile([P, NT, D], BF16, tag="m")
    x1T = m.tile([P, DT, L], BF16, tag="m")

    def emit_ln1(lt, res, rsq, mrs):
        z = stage.tile([P, D], BF16, tag="zb")
        ln_finish(x1[:, lt, :], res[:, lt, :], rsq[:, lt, :], mrs[:, lt, :],
                  bcast["g1"], bcast["bb1"], via=z)
        nc.sync.dma_start(
            x1T[:, :, lt * P:(lt + 1) * P], x1[:, lt, :], transpose=True
        )

    ln_phase(x_res, sa, bcast["g1"], bcast["bb1"], emit_ln1)
    if stop_after == "x1":
        tap(x1)
        return

    # ================= cross attention =================
    qt_c = project_qk("wq_c", bqk["bqk_c"], 0, x1T, lc_outer=True)
    ca = m.tile([P, NT, D], BF16, tag="m")
    attention(ca, False, qt_c, kt_c, V_c)
    if stop_after == "ca":
        tap(ca)
        return

    # ---- residual + LN2 -> x2 (bf16) and x2T ----
    x2 = m.tile([P, NT, D], BF16, tag="m")
    x2T = m.tile([P, DT, L], BF16, tag="m")

    def emit_ln2(lt, res, rsq, mrs):
        z = stage.tile([P, D], BF16, tag="zb")
        ln_finish(x2[:, lt, :], res[:, lt, :], rsq[:, lt, :], mrs[:, lt, :],
                  bcast["g2"], bcast["bb2"], via=z)
        nc.sync.dma_start(
            x2T[:, :, lt * P:(lt + 1) * P], x2[:, lt, :], transpose=True
        )

    ln_phase(x1, ca, bcast["g2"], bcast["bb2"], emit_ln2)
    if stop_after == "x2":
        tap(x2)
        return

    # ================= FFN (dff quarters) =================
    y_bf = m.tile([P, NT, D], BF16, tag="m")
    res3 = None
    mvall = stage.tile([P, NT, 2], FP32, tag="mv")
    for q in range(FQ):
        w1 = m.tile([P, DT, FT * P], BF16, tag="m")
        nc.scalar.dma_start(
            w1,
            ap("w1")[:, q * FT * P:(q + 1) * FT * P].rearrange(
                "(dt p) c -> p dt c", p=P
            ),
        )
        h1 = m.tile([P, FT, L], BF16, tag="m")
        if q == 0:
            # lc-outer: h1 for the first l-half only needs x2T's first 512
            # l-columns (LN2 blocks 0..3) -> FFN starts during LN2.
            for lc in range(NLC):
                for ft in range(FT):
                    ps = psum_sc.tile([P, 1, LC], FP32, tag="sc")
                    for dt in range(DT):
                        nc.tensor.matmul(
                            ps[:, 0, :],
                            w1[:, dt, ft * P:(ft + 1) * P],
                            x2T[:, dt, lc * LC:(lc + 1) * LC],
                            start=(dt == 0),
                            stop=(dt == DT - 1),
                        )
                    nc.scalar.activation(
                        h1[:, ft, lc * LC:(lc + 1) * LC],
                        ps[:, 0, :],
                        AF.Relu,
                        bias=b1_col[:, q * FT + ft:q * FT + ft + 1],
                    )
        else:
            for ft in range(FT):
                ps = psum_sc.tile([P, NLC, LC], FP32, tag="sc")
                for dt in range(DT):
                    lhsT = w1[:, dt, ft * P:(ft + 1) * P]
                    for lc in range(NLC):
                        nc.tensor.matmul(
                            ps[:, lc, :],
                            lhsT,
                            x2T[:, dt, lc * LC:(lc + 1) * LC],
                            start=(dt == 0),
                            stop=(dt == DT - 1),
                        )
                nc.scalar.activation(
                    h1[:, ft, :].rearrange("p (a b) -> p a b", a=NLC),
                    ps,
                    AF.Relu,
                    bias=b1_col[:, q * FT + ft:q * FT + ft + 1],
                )
        w2 = m.tile([P, FT, D], BF16, tag="m")
        nc.scalar.dma_start(
            w2,
            ap("w2")[q * FT * P:(q + 1) * FT * P, :].rearrange(
                "(ft p) c -> p ft c", p=P
            ),
        )
        if q == FQ - 1:
            res3 = m.tile([P, NT, D], BF16, tag="m")
        for lb in range(NT):
            ps = psum_pj.tile([P, NLC, LC], FP32, tag="pj")
            for ft in range(FT):
                lhsT = h1[:, ft, lb * P:(lb + 1) * P]
                for lc in range(NLC):
                    nc.tensor.matmul(
                        ps[:, lc, :],
                        lhsT,
                        w2[:, ft, lc * LC:(lc + 1) * LC],
                        start=(ft == 0),
                        stop=(ft == FT - 1),
                    )
            psv = ps.rearrange("p a b -> p (a b)")
            if q == 0:
                nc.vector.tensor_add(y_bf[:, lb, :], psv, bcast["b2"])
            elif q < FQ - 1:
                nc.vector.tensor_add(y_bf[:, lb, :], y_bf[:, lb, :], psv)
            else:
                t = stage.tile([P, D], BF16, tag="zb")
                nc.vector.tensor_add(t, y_bf[:, lb, :], psv)
                nc.vector.tensor_add(res3[:, lb, :], t, x2[:, lb, :])
                st = stage.tile([P, 2, 6], FP32, tag="bnst")
                nc.vector.bn_stats(st[:, 0, :], res3[:, lb, 0:512])
                nc.vector.bn_stats(st[:, 1, :], res3[:, lb, 512:1024])
                nc.vector.bn_aggr(mvall[:, lb, :], st)

    # ---- final LN (reuses ln2 params), fp32 out, per 4-block half ----
    rsq = stage.tile([P, NT, 1], FP32, tag="rsq")
    mrs = stage.tile([P, NT, 1], FP32, tag="mrs")
    for hf in range(2):
        sl = slice(hf * 4, hf * 4 + 4)
        sq = stage.tile([P, 4, 1], FP32, tag="sq")
        nc.scalar.activation(sq, mvall[:, sl, 1:2], AF.Sqrt, bias=eps_t[:, 0:1])
        nc.vector.reciprocal(rsq[:, sl, :], sq)
        nc.vector.tensor_mul(mrs[:, sl, :], mvall[:, sl, 0:1], rsq[:, sl, :])
        for lt in range(hf * 4, hf * 4 + 4):
            o = stage.tile([P, D], FP32, tag="zf")
            ln_finish(o, res3[:, lt, :], rsq[:, lt, :], mrs[:, lt, :],
                      bcast["g2"], bcast["bb2"])
            nc.sync.dma_start(out_d.ap()[lt * P:(lt + 1) * P, :], o)


_NC_CACHE = {}


def _get_nc(stop_after=None):
    key = stop_after
    if key not in _NC_CACHE:
        _NC_CACHE[key] = build(stop_after)
    return _NC_CACHE[key]


def _pack_weights(inputs):
    """Host-side prepack: cast to bf16 and lay out as the kernel expects."""
    f32 = lambda k: np.ascontiguousarray(np.asarray(inputs[k], dtype=np.float32))
    bf = lambda a: np.ascontiguousarray(np.asarray(a, dtype=NP_BF16))

    def attn_w(k):
        # [H, D, DK] -> [D, H*DK] bf16
        w = f32(k).transpose(1, 0, 2).reshape(D, H * DK)
        return bf(w)

    return {
        "wq_m": attn_w("m_wq"), "wk_m": attn_w("m_wk"), "wv_m": attn_w("m_wv"),
        "wq_c": attn_w("c_wq"), "wk_c": attn_w("c_wk"), "wv_c": attn_w("c_wv"),
        "bqk_m": np.ascontiguousarray(
            np.stack([f32("m_bq").reshape(-1), f32("m_bk").reshape(-1)])
        ),
        "bqk_c": np.ascontiguousarray(
            np.stack([f32("c_bq").reshape(-1), f32("c_bk").reshape(-1)])
        ),
        "bv_m": bf(f32("m_bv").reshape(-1)),
        "bv_c": bf(f32("c_bv").reshape(-1)),
        "w1": bf(f32("ff_w1")),
        "w2": bf(f32("ff_w2")),
        "b1": f32("ff_b1"),
        "b2": bf(f32("ff_b2")),
        "g1": bf(f32("ln1_g")), "bb1": bf(f32("ln1_b")),
        "g2": bf(f32("ln2_g")), "bb2": bf(f32("ln2_b")),
    }


def _make_in_maps(inputs):
    xs = np.ascontiguousarray(
        np.asarray(inputs["decoder_embedding"], dtype=np.float32).astype(NP_BF16)
    )
    es = np.ascontiguousarray(
        np.asarray(inputs["encoder_output"], dtype=np.float32).astype(NP_BF16)
    )
    packed = _pack_weights(inputs)
    return [{**packed, "x": xs[b], "enc": es[b]} for b in range(B)]


def _gather(res):
    return np.stack([res.results[b]["out"] for b in range(B)], axis=0).astype(np.float32)


def kernel(**inputs):
    nc = _get_nc()
    res = run_bass_kernel_spmd(nc, _make_in_maps(inputs), core_ids=list(range(B)))
    return _gather(res)


# revision 28
# speedup vs baseline: 13582.9059x; 1.0115x over previous
"""Transformer decoder layer (masked self-attn + cross-attn + FFN, 3x LayerNorm)
for Trainium2, data-parallel over batch across 8 NeuronCores.

Per-core problem: L=1024 tokens, D=1024 model dim, H=16 heads x 64, DFF=4096.

v2 design (prior baseline measured 1.79 ms/core on the NTFF profile):
- Host prepacks everything to bf16 in SBUF-ready layouts; the kernel DMAs
  weights/activations straight into place (no on-device fp32 staging/casts).
- Blocked DMA_TRANSPOSE: one instruction per [1024,1024] matrix (DRAM source)
  or per [128,1024] row-block (SBUF source) instead of per-128x128 tile.
- V is projected directly into [key-token, h*65] layout (stationary = x^T
  chunk, moving = wv) with a ones column per head, so the O^T matmul also
  emits the softmax denominator; V needs no transposes at all.
- S^T = K^T.T @ Q^T per head, two heads packed into the PE via row groups.
- exp on ScalarE over 2-bank PSUM groups; per-phase batched Rsqrt for the
  LayerNorms (one ACT instruction per LN phase -> no table-set thrash).
- FFN uses h1 chunks as the stationary operand so y lands untransposed.
- SBUF: one shared pool of 10 x ~16.6KB slots (tag "m") recycled across
  phases + small pools; PSUM: 2x[128,2,512] "sc" + 2x[128,2,512] "pj".
"""

import sys

sys.path.insert(0, "/opt/trn_rl_repo")

import numpy as np
import ml_dtypes

import concourse.bass as bass
import concourse.mybir as mybir
import concourse.tile as tile
from concourse import bacc
from concourse.bass_utils import run_bass_kernel_spmd

FP32 = mybir.dt.float32
BF16 = mybir.dt.bfloat16
AF = mybir.ActivationFunctionType
ALU = mybir.AluOpType

B = 8
L = 1024
D = 1024
H = 16
DK = 64
DFF = 4096
P = 128
NT = L // P  # 8 l-tiles
DT = D // P  # 8 d-tiles
NP = H // 2  # 8 head pairs
LC = 512
NLC = L // LC  # 2
FQ = 4  # ffn dff quarters
FT = DFF // FQ // P  # 8 f-tiles per quarter
EPS = 1e-5

NP_BF16 = ml_dtypes.bfloat16

INPUT_SPECS = {
    "x": ([L, D], BF16),
    "enc": ([L, D], BF16),
    "wq_m": ([D, D], BF16), "wk_m": ([D, D], BF16), "wv_m": ([D, D], BF16),
    "wq_c": ([D, D], BF16), "wk_c": ([D, D], BF16), "wv_c": ([D, D], BF16),
    "bqk_m": ([2, D], FP32), "bqk_c": ([2, D], FP32),
    "bv_m": ([D], BF16), "bv_c": ([D], BF16),
    "w1": ([D, DFF], BF16), "w2": ([DFF, D], BF16),
    "b1": ([DFF], FP32), "b2": ([D], BF16),
    "g1": ([D], BF16), "bb1": ([D], BF16),
    "g2": ([D], BF16), "bb2": ([D], BF16),
}


def _bcast_ap(ap, parts=P):
    """Broadcast a 1-D DRAM AP across `parts` partitions (step-0 partition dim)."""
    return bass.AP(tensor=ap.tensor, offset=ap.offset, ap=[[0, parts]] + list(ap.ap))


def build(stop_after=None):
    nc = bacc.Bacc("TRN2", target_bir_lowering=False, debug=False, num_devices=B)

    dram = {}
    for name, (shape, dt) in INPUT_SPECS.items():
        dram[name] = nc.dram_tensor(name, shape, dt, kind="ExternalInput")
    out_d = nc.dram_tensor("out", [L, D], FP32, kind="ExternalOutput")

    with tile.TileContext(nc) as tc:
        _emit(nc, tc, dram, out_d, stop_after)
    nc.compile()
    return nc


def _emit(nc, tc, dram, out_d, stop_after):
    with tc.tile_pool(name="const", bufs=1) as const, \
         tc.tile_pool(name="m", bufs=10) as m, \
         tc.tile_pool(name="heads", bufs=3) as heads, \
         tc.tile_pool(name="stage", bufs=2, side="left") as stage, \
         tc.tile_pool(name="pj", bufs=2, space=bass.MemorySpace.PSUM) as psum_pj, \
         tc.tile_pool(name="sc", bufs=2, space=bass.MemorySpace.PSUM) as psum_sc:
        _body(nc, dram, out_d, stop_after, const, m, heads, stage, psum_pj, psum_sc)


def _body(nc, dram, out_d, stop_after, const, m, heads, stage, psum_pj, psum_sc):
    def ap(name):
        return dram[name].ap()

    # ---- constants ----
    eps_t = const.tile([P, 1], FP32)
    nc.vector.memset(eps_t, EPS)

    # causal 0/1 masks for diagonal blocks: mask[i][kk, qq] = 1 if qq >= kk + i*128
    mask_bf = const.tile([P, 4, LC], BF16)
    for i in range(4):
        m32 = stage.tile([P, LC], FP32, tag="zb")
        nc.vector.memset(m32, 1.0)
        nc.gpsimd.affine_select(
            out=m32,
            in_=m32,
            compare_op=ALU.is_ge,
            fill=0.0,
            base=-(i * P),
            pattern=[[1, LC]],
            channel_multiplier=-1,
        )
        nc.vector.tensor_copy(mask_bf[:, i, :], m32)

    # ---- inputs (xT first: the first projection waits on it) ----
    xT = m.tile([P, DT, L], BF16, tag="m")
    for lh in range(2):
        nc.sync.dma_start(
            xT[:, :, lh * LC:(lh + 1) * LC],
            ap("x")[lh * LC:(lh + 1) * LC, :],
            transpose=True,
        )
    encT = m.tile([P, DT, L], BF16, tag="m")
    nc.sync.dma_start(encT, ap("enc"), transpose=True)

    # per-partition bias columns for Q/K projections: [128, 2(j=q/k), 8(pr)]
    bqk = {}
    for name in ("bqk_m", "bqk_c"):
        t = const.tile([P, 2, NP], FP32, tag=name)
        nc.gpsimd.dma_start(t, ap(name).rearrange("j (pr p) -> p j pr", p=P))
        bqk[name] = t
    # per-partition bias for FFN1 relu: [128, 32]
    b1_col = const.tile([P, DFF // P], FP32)
    nc.gpsimd.dma_start(b1_col, ap("b1").rearrange("(ft p) -> p ft", p=P))
    # broadcast (all-partition) bias/param rows, bf16 [128, 1024]
    bcast = {}
    for name in ("bv_m", "bv_c", "b2", "g1", "bb1", "g2", "bb2"):
        t = const.tile([P, D], BF16, tag=f"bc_{name}")
        nc.gpsimd.dma_start(t, _bcast_ap(ap(name)))
        bcast[name] = t

    # ---- helpers ----
    # Weight/param loads go through SWDGE (gpsimd): HWDGE rings serialize
    # against xbar-transpose mode switches, so plain loads there stall behind
    # every DMA_TRANSPOSE in flight. SWDGE rings do not.
    def load_w(name):
        w = m.tile([P, DT, D], BF16, tag="m")
        nc.gpsimd.dma_start(w, ap(name).rearrange("(dt p) c -> p dt c", p=P))
        return w

    def project_qk(wname, b_col, j, srcT, lc_outer=False):
        # returns [128(i*64+k), NP, L]: per head-pair column block of W^T srcT + b
        # lc_outer: emit all head-pairs for l-chunk 0 first so consumers of the
        # first chunk (and producers of only the first srcT l-columns) pipeline.
        dst = m.tile([P, NP, L], BF16, tag="m")
        w = load_w(wname)
        if lc_outer:
            for lc in range(NLC):
                for pr in range(NP):
                    ps = psum_pj.tile([P, 1, LC], FP32, tag="pj")
                    for dt in range(DT):
                        nc.tensor.matmul(
                            ps[:, 0, :],
                            w[:, dt, pr * P:(pr + 1) * P],
                            srcT[:, dt, lc * LC:(lc + 1) * LC],
                            start=(dt == 0),
                            stop=(dt == DT - 1),
                        )
                    nc.vector.tensor_scalar_add(
                        dst[:, pr, lc * LC:(lc + 1) * LC],
                        ps[:, 0, :],
                        b_col[:, j, pr:pr + 1],
                    )
            return dst
        for pr in range(NP):
            ps = psum_pj.tile([P, NLC, LC], FP32, tag="pj")
            for dt in range(DT):
                lhsT = w[:, dt, pr * P:(pr + 1) * P]
                for lc in range(NLC):
                    nc.tensor.matmul(
                        ps[:, lc, :],
                        lhsT,
                        srcT[:, dt, lc * LC:(lc + 1) * LC],
                        start=(dt == 0),
                        stop=(dt == DT - 1),
                    )
            nc.vector.tensor_scalar_add(
                dst[:, pr, :].rearrange("p (a b) -> p a b", a=NLC),
                ps,
                b_col[:, j, pr:pr + 1],
            )
        return dst

    def project_v(wname, bv_bc, srcT):
        # V [128(lk), NT, H*65]: V[:, kt, h*65+v] = (srcT_chunk.T @ wv)[lk, h*64+v] + bv
        # col 65*h+64 is a ones column.
        V = m.tile([P, NT, H * 65], BF16, tag="m")
        w = load_w(wname)
        for kt in range(NT):
            ps = psum_pj.tile([P, NLC, LC], FP32, tag="pj")
            for dt in range(DT):
                lhsT = srcT[:, dt, kt * P:(kt + 1) * P]
                for lc in range(NLC):
                    nc.tensor.matmul(
                        ps[:, lc, :],
                        lhsT,
                        w[:, dt, lc * LC:(lc + 1) * LC],
                        start=(dt == 0),
                        stop=(dt == DT - 1),
                    )
            Vv = V[:, kt, :].rearrange("p (h c) -> p h c", c=65)
            for lc in range(NLC):
                nc.vector.tensor_add(
                    Vv[:, lc * 8:(lc + 1) * 8, 0:64],
                    ps[:, lc, :].rearrange("p (h c) -> p h c", c=64),
                    bv_bc[:, lc * LC:(lc + 1) * LC].rearrange(
                        "p (h c) -> p h c", c=64
                    ),
                )
        nc.vector.memset(
            V.rearrange("p a (h c) -> p a h c", c=65)[:, :, :, 64:65], 1.0
        )
        return V

    def attention(out_sa, causal, qt, kt, V):
        for pr in range(NP):
            eS0 = m.tile([P, NT, L], BF16, tag="m")
            eS1 = m.tile([P, NT, L], BF16, tag="m")
            eS = [eS0, eS1]
            for lc in range(NLC):
                kts = list(range(4)) if (causal and lc == 0) else list(range(NT))
                for g0 in range(0, len(kts), 2):
                    grp = kts[g0:g0 + 2]
                    ps0 = psum_sc.tile([P, 2, LC], FP32, tag="sc")
                    ps1 = psum_sc.tile([P, 2, LC], FP32, tag="sc")
                    pss = [ps0, ps1]
                    for j, kt_ in enumerate(grp):
                        for i in range(2):
                            r0 = i * 64
                            nc.tensor.matmul(
                                pss[i][:, j, :],
                                kt[r0:r0 + 64, pr, kt_ * P:(kt_ + 1) * P],
                                qt[r0:r0 + 64, pr, lc * LC:(lc + 1) * LC],
                                start=True,
                                stop=True,
                                tile_position=(r0, 0),
                            )
                    for i in range(2):
                        nc.scalar.activation(
                            eS[i][:, grp[0]:grp[0] + len(grp),
                                  lc * LC:(lc + 1) * LC],
                            pss[i][:, 0:len(grp), :],
                            AF.Exp,
                            scale=0.125,
                        )
                    if causal:
                        for kt_ in grp:
                            if kt_ >= 4 * lc:
                                mi = kt_ - 4 * lc
                                for i in range(2):
                                    nc.vector.tensor_mul(
                                        eS[i][:, kt_, lc * LC:(lc + 1) * LC],
                                        eS[i][:, kt_, lc * LC:(lc + 1) * LC],
                                        mask_bf[:, mi, :],
                                    )
            # O^T rows 0:64 + softmax denominator row 64 (ones column of V)
            for i in range(2):
                h = 2 * pr + i
                av = psum_pj.tile([P, NLC, LC], FP32, tag="pj")
                for lc in range(NLC):
                    kts = list(range(4)) if (causal and lc == 0) else list(range(NT))
                    for j, kt_ in enumerate(kts):
                        nc.tensor.matmul(
                            av[0:65, lc, :],
                            V[:, kt_, h * 65:h * 65 + 65],
                            eS[i][:, kt_, lc * LC:(lc + 1) * LC],
                            start=(j == 0),
                            stop=(j == len(kts) - 1),
                        )
                ot = heads.tile([80, L], BF16, tag="ot")
                nc.vector.memset(ot[64:80, :], 0.0)
                nc.vector.tensor_copy(
                    ot[0:65, :].rearrange("p (a b) -> p a b", a=NLC), av[0:65]
                )
                otr = heads.tile([P, NT, 80], BF16, tag="otr")
                nc.sync.dma_start(otr, ot, transpose=True)
                rcp = heads.tile([P, NT, 1], FP32, tag="rcp")
                nc.vector.reciprocal(rcp, otr[:, :, 64:65])
                nc.vector.tensor_mul(
                    out_sa.rearrange("p lt (hh c) -> p lt hh c", c=64)[:, :, h, :],
                    otr[:, :, 0:64],
                    rcp.broadcast_to([P, NT, 64]),
                )

    def ln_stats_block(res_lt, sums_lt, ssq_lt):
        # Sigma r^2 on ScalarE (runs parallel to the DVE chain)
        dump = stage.tile([P, D], FP32, tag="zf")
        nc.scalar.activation(dump, res_lt, AF.Square, accum_out=ssq_lt)

    def ln_half_scalars(sums, ssq, rsq, mrs, sl):
        # mean = sums/D; var = ssq/D - mean^2; rstd = 1/sqrt(var+eps)
        mh = stage.tile([P, 4, 1], FP32, tag="mh")
        nc.vector.tensor_scalar_mul(mh, sums[:, sl, :], 1.0 / D)
        m2 = stage.tile([P, 4, 1], FP32, tag="m2")
        nc.vector.tensor_mul(m2, mh, mh)
        v1 = stage.tile([P, 4, 1], FP32, tag="v1")
        nc.vector.scalar_tensor_tensor(
            v1, ssq[:, sl, :], 1.0 / D, m2, op0=ALU.mult, op1=ALU.subtract
        )
        sq = stage.tile([P, 4, 1], FP32, tag="sq")
        nc.scalar.activation(sq, v1, AF.Sqrt, bias=eps_t[:, 0:1])
        nc.vector.reciprocal(rsq[:, sl, :], sq)
        nc.vector.tensor_mul(mrs[:, sl, :], mh, rsq[:, sl, :])

    def ln_phase(a_big, b_big, g_t, b_t, emit_block, res_name="res"):
        # residual r = a+b with free-dim sum accumulated in the same DVE op;
        # stats batched per half (4 blocks) so downstream work starts early.
        res = m.tile([P, NT, D], BF16, tag="m")
        sums = stage.tile([P, NT, 1], FP32, tag="sums")
        ssq = stage.tile([P, NT, 1], FP32, tag="ssq")
        rsq = stage.tile([P, NT, 1], FP32, tag="rsq")
        mrs = stage.tile([P, NT, 1], FP32, tag="mrs")
        for hf in range(2):
            lts = range(hf * 4, hf * 4 + 4)
            for lt in lts:
                nc.vector.scalar_tensor_tensor(
                    res[:, lt, :], a_big[:, lt, :], 1.0, b_big[:, lt, :],
                    op0=ALU.mult, op1=ALU.add, accum_out=sums[:, lt, :],
                )
                ln_stats_block(res[:, lt, :], sums[:, lt, :], ssq[:, lt, :])
            sl = slice(hf * 4, hf * 4 + 4)
            ln_half_scalars(sums, ssq, rsq, mrs, sl)
            for lt in lts:
                emit_block(lt, res, rsq, mrs)
        return res

    def ln_finish(dst, res_lt, rsq_lt, mrs_lt, g_t, b_t, via=None):
        z = via if via is not None else dst
        nc.vector.tensor_scalar(
            z, res_lt, rsq_lt, mrs_lt, op0=ALU.mult, op1=ALU.subtract
        )
        nc.vector.tensor_mul(dst, z, g_t)
        nc.vector.tensor_add(dst, dst, b_t)

    def tap(src_big):
        for lt in range(NT):
            o = stage.tile([P, D], FP32, tag="zf")
            nc.vector.tensor_copy(o, src_big[:, lt, :])
            nc.sync.dma_start(out_d.ap()[lt * P:(lt + 1) * P, :], o)

    # ================= self attention =================
    qt_s = project_qk("wq_m", bqk["bqk_m"], 0, xT)
    kt_s = project_qk("wk_m", bqk["bqk_m"], 1, xT)
    V_s = project_v("wv_m", bcast["bv_m"], xT)
    # xT's slot is recycled after V_s projection (last reader)

    sa = m.tile([P, NT, D], BF16, tag="m")
    attention(sa, True, qt_s, kt_s, V_s)
    if stop_after == "sa":
        tap(sa)
        return

    # cross K/V projections (can fill PE gaps at the tail of self-attn)
    x_res = m.tile([P, NT, D], BF16, tag="m")
    nc.gpsimd.dma_start(x_res, ap("x").rearrange("(lt p) d -> p lt d", p=P))
    kt_c = project_qk("wk_c", bqk["bqk_c"], 1, encT)
    V_c = project_v("wv_c", bcast["bv_c"], encT)

    # ---- residual + LN1 -> x1 (bf16) and x1T ----
    x1 = m.tile([P, NT, D], BF16, tag="m")
    x1T = m.tile([P, DT, L], BF16, tag="m")

    def emit_ln1(lt, res, rsq, mrs):
        z = stage.tile([P, D], BF16, tag="zb")
        ln_finish(x1[:, lt, :], res[:, lt, :], rsq[:, lt, :], mrs[:, lt, :],
                  bcast["g1"], bcast["bb1"], via=z)
        nc.sync.dma_start(
            x1T[:, :, lt * P:(lt + 1) * P], x1[:, lt, :], transpose=True
        )

    ln_phase(x_res, sa, bcast["g1"], bcast["bb1"], emit_ln1)
    if stop_after == "x1":
        tap(x1)
        return

    # ================= cross attention =================
    qt_c = project_qk("wq_c", bqk["bqk_c"], 0, x1T, lc_outer=True)
    ca = m.tile([P, NT, D], BF16, tag="m")
    attention(ca, False, qt_c, kt_c, V_c)
    if stop_after == "ca":
        tap(ca)
        return

    # ---- residual + LN2 -> x2 (bf16) and x2T ----
    x2 = m.tile([P, NT, D], BF16, tag="m")
    x2T = m.tile([P, DT, L], BF16, tag="m")

    def emit_ln2(lt, res, rsq, mrs):
        z = stage.tile([P, D], BF16, tag="zb")
        ln_finish(x2[:, lt, :], res[:, lt, :], rsq[:, lt, :], mrs[:, lt, :],
                  bcast["g2"], bcast["bb2"], via=z)
        nc.sync.dma_start(
            x2T[:, :, lt * P:(lt + 1) * P], x2[:, lt, :], transpose=True
        )

    ln_phase(x1, ca, bcast["g2"], bcast["bb2"], emit_ln2)
    if stop_after == "x2":
        tap(x2)
        return

    # ================= FFN (dff quarters) =================
    y_bf = m.tile([P, NT, D], BF16, tag="m")
    res3 = None
    sums3 = stage.tile([P, NT, 1], FP32, tag="sums")
    ssq3 = stage.tile([P, NT, 1], FP32, tag="ssq")
    for q in range(FQ):
        w1 = m.tile([P, DT, FT * P], BF16, tag="m")
        nc.gpsimd.dma_start(
            w1,
            ap("w1")[:, q * FT * P:(q + 1) * FT * P].rearrange(
                "(dt p) c -> p dt c", p=P
            ),
        )
        h1 = m.tile([P, FT, L], BF16, tag="m")
        if q == 0:
            # lc-outer: h1 for the first l-half only needs x2T's first 512
            # l-columns (LN2 blocks 0..3) -> FFN starts during LN2.
            for lc in range(NLC):
                for ft in range(FT):
                    ps = psum_sc.tile([P, 1, LC], FP32, tag="sc")
                    for dt in range(DT):
                        nc.tensor.matmul(
                            ps[:, 0, :],
                            w1[:, dt, ft * P:(ft + 1) * P],
                            x2T[:, dt, lc * LC:(lc + 1) * LC],
                            start=(dt == 0),
                            stop=(dt == DT - 1),
                        )
                    nc.scalar.activation(
                        h1[:, ft, lc * LC:(lc + 1) * LC],
                        ps[:, 0, :],
                        AF.Relu,
                        bias=b1_col[:, q * FT + ft:q * FT + ft + 1],
                    )
        else:
            for ft in range(FT):
                ps = psum_sc.tile([P, NLC, LC], FP32, tag="sc")
                for dt in range(DT):
                    lhsT = w1[:, dt, ft * P:(ft + 1) * P]
                    for lc in range(NLC):
                        nc.tensor.matmul(
                            ps[:, lc, :],
                            lhsT,
                            x2T[:, dt, lc * LC:(lc + 1) * LC],
                            start=(dt == 0),
                            stop=(dt == DT - 1),
                        )
                nc.scalar.activation(
                    h1[:, ft, :].rearrange("p (a b) -> p a b", a=NLC),
                    ps,
                    AF.Relu,
                    bias=b1_col[:, q * FT + ft:q * FT + ft + 1],
                )
        w2 = m.tile([P, FT, D], BF16, tag="m")
        nc.gpsimd.dma_start(
            w2,
            ap("w2")[q * FT * P:(q + 1) * FT * P, :].rearrange(
                "(ft p) c -> p ft c", p=P
            ),
        )
        if q == FQ - 1:
            res3 = m.tile([P, NT, D], BF16, tag="m")
        for lb in range(NT):
            ps = psum_pj.tile([P, NLC, LC], FP32, tag="pj")
            for ft in range(FT):
                lhsT = h1[:, ft, lb * P:(lb + 1) * P]
                for lc in range(NLC):
                    nc.tensor.matmul(
                        ps[:, lc, :],
                        lhsT,
                        w2[:, ft, lc * LC:(lc + 1) * LC],
                        start=(ft == 0),
                        stop=(ft == FT - 1),
                    )
            psv = ps.rearrange("p a b -> p (a b)")
            if q == 0:
                nc.vector.tensor_add(y_bf[:, lb, :], psv, bcast["b2"])
            elif q < FQ - 1:
                nc.vector.tensor_add(y_bf[:, lb, :], y_bf[:, lb, :], psv)
            else:
                t = stage.tile([P, D], BF16, tag="zb")
                nc.vector.tensor_add(t, y_bf[:, lb, :], psv)
                nc.vector.scalar_tensor_tensor(
                    res3[:, lb, :], t, 1.0, x2[:, lb, :],
                    op0=ALU.mult, op1=ALU.add, accum_out=sums3[:, lb, :],
                )
                ln_stats_block(res3[:, lb, :], sums3[:, lb, :], ssq3[:, lb, :])

    # ---- final LN (reuses ln2 params), fp32 out, per 4-block half ----
    rsq = stage.tile([P, NT, 1], FP32, tag="rsq")
    mrs = stage.tile([P, NT, 1], FP32, tag="mrs")
    for hf in range(2):
        sl = slice(hf * 4, hf * 4 + 4)
        ln_half_scalars(sums3, ssq3, rsq, mrs, sl)
        for lt in range(hf * 4, hf * 4 + 4):
            o = stage.tile([P, D], FP32, tag="zf")
            ln_finish(o, res3[:, lt, :], rsq[:, lt, :], mrs[:, lt, :],
                      bcast["g2"], bcast["bb2"])
            nc.sync.dma_start(out_d.ap()[lt * P:(lt + 1) * P, :], o)


_NC_CACHE = {}


def _get_nc(stop_after=None):
    key = stop_after
    if key not in _NC_CACHE:
        _NC_CACHE[key] = build(stop_after)
    return _NC_CACHE[key]


def _pack_weights(inputs):
    """Host-side prepack: cast to bf16 and lay out as the kernel expects."""
    f32 = lambda k: np.ascontiguousarray(np.asarray(inputs[k], dtype=np.float32))
    bf = lambda a: np.ascontiguousarray(np.asarray(a, dtype=NP_BF16))

    def attn_w(k):
        # [H, D, DK] -> [D, H*DK] bf16
        w = f32(k).transpose(1, 0, 2).reshape(D, H * DK)
        return bf(w)

    return {
        "wq_m": attn_w("m_wq"), "wk_m": attn_w("m_wk"), "wv_m": attn_w("m_wv"),
        "wq_c": attn_w("c_wq"), "wk_c": attn_w("c_wk"), "wv_c": attn_w("c_wv"),
        "bqk_m": np.ascontiguousarray(
            np.stack([f32("m_bq").reshape(-1), f32("m_bk").reshape(-1)])
        ),
        "bqk_c": np.ascontiguousarray(
            np.stack([f32("c_bq").reshape(-1), f32("c_bk").reshape(-1)])
        ),
        "bv_m": bf(f32("m_bv").reshape(-1)),
        "bv_c": bf(f32("c_bv").reshape(-1)),
        "w1": bf(f32("ff_w1")),
        "w2": bf(f32("ff_w2")),
        "b1": f32("ff_b1"),
        "b2": bf(f32("ff_b2")),
        "g1": bf(f32("ln1_g")), "bb1": bf(f32("ln1_b")),
        "g2": bf(f32("ln2_g")), "bb2": bf(f32("ln2_b")),
    }


def _make_in_maps(inputs):
    xs = np.ascontiguousarray(
        np.asarray(inputs["decoder_embedding"], dtype=np.float32).astype(NP_BF16)
    )
    es = np.ascontiguousarray(
        np.asarray(inputs["encoder_output"], dtype=np.float32).astype(NP_BF16)
    )
    packed = _pack_weights(inputs)
    return [{**packed, "x": xs[b], "enc": es[b]} for b in range(B)]


def _gather(res):
    return np.stack([res.results[b]["out"] for b in range(B)], axis=0).astype(np.float32)


def kernel(**inputs):
    nc = _get_nc()
    res = run_bass_kernel_spmd(nc, _make_in_maps(inputs), core_ids=list(range(B)))
    return _gather(res)
